# revision 36
# baseline (speedup 1.0000x reference)
"""Trainium2 Bass kernel for masked-GRU + residual + LayerNorm.

Problem: N=128 sequences of length L=512, hidden H=512.
  gx = x @ W_ih.T + b_ih            (precomputable input projection)
  per step l: hc = h * (1-is_initial[l]); gh = hc @ W_hh.T + b_hh
    r = sig(gx_r+gh_r); z = sig(gx_z+gh_z); n = tanh(gx_n + r*gh_n)
    h = (1-z)*n + z*hc
  out = LayerNorm(seq + x) * gamma + beta;  h_exp = broadcast(h_last)

Strategy (v2):
  * Data parallel: 16 batch rows per core (8 cores).
  * Sequence-chunk parallel: each L=512 sequence is split into C=16
    chunks of 32 steps, processed as independent columns, made exact by
    an R-step warm-up (a reset lands inside every R-window; verified at
    runtime). Chunk 0 injects the true h0 at entry to main.
  * bf16 everywhere off-PSUM: weights, x, gates, state, y, outputs.
    PSUM stays f32.  DVE runs 2-byte all-SBUF ops at 4x rate.
  * Packed state s [128, 1024] bf16 (4 h-tiles side by side in the free
    dim) so the elementwise chain runs as [128,512] halves.
  * PSUM banks (8): R pair (r gates), Z pair (z), G pair (gh_n),
    X pair (gx_n, lives across one step boundary).  LN stats borrow
    bank G0 right after it drains.
  * n-gate: gx_n stays in PSUM; w=(gh+bhh_n)*r on DVE, u=w+gx on Pool,
    tanh(u)+b_ih_n on ACT.  No identity-drain of gx.
  * LayerNorm per block of 8 steps, software-pipelined over the 7
    following steps so nothing blocks the recurrence: column sums via
    indicator-stationary matmuls into ONE [8,512] PSUM tile (rows 0-3
    Smu, rows 4-7 Sss), stats math as [4,512] multi-partition ops
    (D = H*Sss - Smu^2; rs = 1/sqrt(D + eps*H^2); out = ((H*y - Smu)
    * rs) * gamma + beta), DRAM-bounce broadcast of (Smu, rs) in bf16.
  * out / h_last stored bf16, upcast host-side.
"""
import sys

sys.path.insert(0, "/opt/trn_rl_repo")

import numpy as np

import concourse.bass as bass
import concourse.tile as tile
from concourse import bacc, mybir
from concourse.bass_utils import run_bass_kernel_spmd

F32 = mybir.dt.float32
BF16 = mybir.dt.bfloat16
AF = mybir.ActivationFunctionType
ALU = mybir.AluOpType

N, L, H = 128, 512, 512
NCORES = 8
NB = N // NCORES          # batch rows per core = 16
C = 16                    # chunks per sequence
KS = L // C               # main steps per chunk = 32
S = NB * C                # columns per core = 256
HT = H // 128             # h partition tiles = 4
BLK = 8                   # LN block (main steps)
NBLK = KS // BLK          # 4
FB = BLK * S              # 2048 block columns
SF = HT * S               # 1024 free dim of the packed state


def _bcast_ap(row_ap, parts=128, rep=1):
    """DRAM row AP -> partition-broadcast AP (0-stride over partitions,
    optionally replicated rep times along free)."""
    ap = [[0, parts]]
    if rep > 1:
        ap.append([0, rep])
    ap += [list(d) for d in row_ap.ap]
    return bass.AP(tensor=row_ap.tensor, offset=row_ap.offset, ap=ap)


def build_program(R=12, debug=False, triv_gb=True):
    T = R + KS
    nc = bacc.Bacc("TRN2", target_bir_lowering=False)

    xs_d = nc.declare_dram_parameter("xs", [T, 128, SF], BF16, isOutput=False)
    ms_d = nc.declare_dram_parameter("ms", [T, S], BF16, isOutput=False)
    h0m_d = nc.declare_dram_parameter("h0m", [128, HT * NB], BF16, isOutput=False)
    wih_d = nc.declare_dram_parameter("wih", [HT, 128, 3 * H], BF16, isOutput=False)
    whh_d = nc.declare_dram_parameter("whh", [HT, 128, 3 * H], BF16, isOutput=False)
    brz_d = nc.declare_dram_parameter("brz", [128, 8], F32, isOutput=False)
    bhn_d = nc.declare_dram_parameter("bhn", [128, HT], F32, isOutput=False)
    bin_d = nc.declare_dram_parameter("bin", [128, HT], F32, isOutput=False)
    gam_d = nc.declare_dram_parameter("gam", [128, HT], F32, isOutput=False)
    bet_d = nc.declare_dram_parameter("bet", [128, HT], F32, isOutput=False)
    ind_d = nc.declare_dram_parameter("ind", [128, 16], BF16, isOutput=False)
    sel_d = nc.declare_dram_parameter("sel", [4, 512], BF16, isOutput=False)

    out_d = nc.declare_dram_parameter("out_st", [HT, 128, KS * S], BF16, isOutput=True)
    if debug:
        dbg_names = ["mk0", "r0", "z0", "n0", "hn0", "s1", "s2x", "hnT"]
        dbg_d = {
            nm: nc.declare_dram_parameter(f"dbg_{nm}", [128, SF], BF16,
                                          isOutput=True)
            for nm in dbg_names
        }
    hl_d = nc.declare_dram_parameter("hlast", [128, HT * NB], BF16, isOutput=True)


    with tile.TileContext(nc) as tc:
        with (
            tc.tile_pool(name="const", bufs=1) as cst,
            tc.tile_pool(name="sb", bufs=1) as sb,
            tc.tile_pool(name="rzp", bufs=1, space="PSUM") as rzp,
            tc.tile_pool(name="gxp", bufs=1, space="PSUM") as gxp,
        ):
            # ---- constants (wih first: needed by the t=0 prefill) ----
            x0 = sb.tile([128, SF], BF16, name="xt0", tag="xt", bufs=3)
            nc.sync.dma_start(out=x0, in_=xs_d[0, :, :])
            wih_sb, whh_sb = [], []
            for k in range(HT):
                w1 = cst.tile([128, 3 * H], BF16, name=f"wih_sb{k}", tag=f"wih{k}")
                (nc.sync if k % 2 else nc.scalar).dma_start(
                    out=w1, in_=wih_d[k, :, :])
                wih_sb.append(w1)
            for k in range(HT):
                w2 = cst.tile([128, 3 * H], BF16, name=f"whh_sb{k}", tag=f"whh{k}")
                (nc.sync if k % 2 else nc.scalar).dma_start(
                    out=w2, in_=whh_d[k, :, :])
                whh_sb.append(w2)
            h0m_sb = cst.tile([128, HT * NB], BF16, name="h0m_sb", tag="h0m")
            nc.sync.dma_start(out=h0m_sb, in_=h0m_d[:, :])
            brz_sb = cst.tile([128, 8], F32, name="brz_sb", tag="brz")
            nc.sync.dma_start(out=brz_sb, in_=brz_d[:, :])
            bhn_sb = cst.tile([128, HT], F32, name="bhn_sb", tag="bhn")
            nc.sync.dma_start(out=bhn_sb, in_=bhn_d[:, :])
            bin_sb = cst.tile([128, HT], F32, name="bin_sb", tag="bin")
            nc.sync.dma_start(out=bin_sb, in_=bin_d[:, :])
            gam_sb = cst.tile([128, HT], F32, name="gam_sb", tag="gam")
            nc.sync.dma_start(out=gam_sb, in_=gam_d[:, :])
            bet_sb = cst.tile([128, HT], F32, name="bet_sb", tag="bet")
            nc.sync.dma_start(out=bet_sb, in_=bet_d[:, :])
            ind_sb = cst.tile([128, 16], BF16, name="ind_sb", tag="ind")
            nc.sync.dma_start(out=ind_sb, in_=ind_d[:, :])
            eps_sb = cst.tile([128, 1], F32, name="eps_sb", tag="eps")
            nc.vector.memset(eps_sb, 1e-5)
            sel_sb = cst.tile([4, 512], BF16, name="sel_sb", tag="sel")
            nc.sync.dma_start(out=sel_sb, in_=sel_d[:, :])
            # masks: load once to partition 0, broadcast on Pool
            ms_all = cst.tile([128, T * S], BF16, name="ms_all", tag="msb")
            nc.sync.dma_start(
                out=ms_all[0:1, :], in_=ms_d[:, :].rearrange("t s -> (t s)"))
            nc.gpsimd.partition_broadcast(ms_all, ms_all[0:1, :])

            # ---- initial (zero) state ----
            s_cur = sb.tile([128, SF], BF16, name="s_init", tag="state", bufs=3)
            nc.vector.memset(s_cur, 0.0)

            def load_x(t):
                xt = sb.tile([128, SF], BF16, name=f"xt{t}", tag="xt", bufs=3)
                nc.sync.dma_start(out=xt, in_=xs_d[t, :, :])
                return xt



            def prefill_gx(t, xt):
                # open+close gx_n groups for step t (wih only); X pair
                # [gx8|gx9], [gx10|gx11]; stays in PSUM until step t's u.
                gx_ps = [
                    gxp.tile([128, 512], F32, name=f"gx{t}_{j}", tag=f"gx{j}")
                    for j in range(2)
                ]
                for k4 in range(4):
                    j = 8 + k4
                    oap = gx_ps[k4 // 2][:, (k4 % 2) * 256 : (k4 % 2) * 256 + 256]
                    for k in range(HT):
                        nc.tensor.matmul(
                            oap, wih_sb[k][:, j * 128 : (j + 1) * 128],
                            xt[:, k * 256 : (k + 1) * 256],
                            start=(k == 0), stop=(k == HT - 1))
                return gx_ps

            xt = x0
            gx_ps = prefill_gx(0, xt)

            # LN pipeline state, keyed by block id
            ln = {}
            y_all = y2_all = None

            for t in range(T):
                main = t >= R
                toff = (t - R) % BLK
                blk = (t - R) // BLK
                blk_end = main and toff == BLK - 1

                # LN pipeline stage for earlier blocks this iteration
                stages = [(t - st["te"], b, st) for b, st in list(ln.items())
                          if 1 <= t - st["te"] <= 8]

                # ---- s1 (ACT top): drain the stats PSUM banks ----
                for dt_, b, st in stages:
                    if dt_ == 1:
                        st_mu = sb.tile([4, 512], F32, name=f"stm{b}",
                                        tag="stm", bufs=2)
                        nc.scalar.activation(
                            out=st_mu, in_=st["stp0"][0:4, :],
                            func=AF.Identity, scale=1.0)
                        st_ss = sb.tile([4, 512], F32, name=f"stv{b}",
                                        tag="stv", bufs=2)
                        nc.scalar.activation(
                            out=st_ss, in_=st["stp1"][0:4, :],
                            func=AF.Identity, scale=1.0)
                        st["st_mu"] = st_mu
                        st["st_ss"] = st_ss

                if t + 1 < T:
                    xt_nxt = load_x(t + 1)
                    mk = ms_all[:, (t + 1) * S : (t + 2) * S]

                # ---- s1 (DVE top): musq, dvar ----
                for dt_, b, st in stages:
                    if dt_ == 1:
                        musq = sb.tile([4, 512], F32, name=f"mq{b}", tag="mq",
                                       bufs=2)
                        nc.vector.tensor_mul(
                            musq, st["st_mu"], st["st_mu"])
                        dvar = sb.tile([4, 512], F32, name=f"dv{b}", tag="dv",
                                       bufs=2)
                        nc.vector.scalar_tensor_tensor(
                            out=dvar, in0=st["st_ss"],
                            scalar=float(H), in1=musq,
                            op0=ALU.mult, op1=ALU.subtract)
                        st["dvar"] = dvar

                # ---- s3 (DVE top): recip + bf16 packs ----
                for dt_, b, st in stages:
                    if dt_ == 3:
                        rst = sb.tile([4, 512], F32, name=f"rs{b}", tag="rs",
                                      bufs=2)
                        nc.vector.reciprocal_approx_fast(out=rst, in_=st["sq"])
                        bsm = sb.tile([4, 512], F32, name=f"bs{b}", tag="bs",
                                      bufs=2)
                        nc.vector.scalar_tensor_tensor(
                            out=bsm, in0=st["st_mu"], scalar=1.0 / float(H),
                            in1=rst, op0=ALU.mult, op1=ALU.mult)
                        pkm = sb.tile([4, 512], BF16, name=f"pkm{b}",
                                      tag="pkm", bufs=2)
                        nc.vector.tensor_copy(pkm, bsm)
                        pkr = sb.tile([4, 512], BF16, name=f"pkr{b}",
                                      tag="pkr", bufs=2)
                        nc.vector.tensor_copy(pkr, rst)
                        st["pkm"] = pkm
                        st["pkr"] = pkr
                        st["mu_bc"] = sb.tile([128, FB], BF16, name=f"mubc{b}",
                                              tag="mubc", bufs=1)
                        st["rs_bc"] = sb.tile([128, FB], BF16, name=f"rsbc{b}",
                                              tag="rsbc", bufs=1)



                # ---- PE: all r/z/gh groups open and close within this
                #      iteration (cross-iteration open groups on sliced
                #      tiles miscompile).  Order: independent wih opens
                #      first, state-dependent whh closes mid-stream, the
                #      complete gx prefill for t+1 last. ----
                r_ps = [
                    rzp.tile([128, 512], F32, name=f"r{t}_{j}", tag=f"r{j}")
                    for j in range(2)
                ]
                gh_ps = [
                    rzp.tile([128, 512], F32, name=f"gh{t}_{j}", tag=f"gh{j}")
                    for j in range(2)
                ]
                z_ps = [
                    rzp.tile([128, 512], F32, name=f"z{t}_{j}", tag=f"z{j}")
                    for j in range(2)
                ]

                def wih_open(ps, j4, j0):
                    # opens the j4 slice group (start zeroes the bank's
                    # write-bitmap: no other start may hit this bank until
                    # this group fully closes)
                    j = j0 + j4
                    oap = ps[j4 // 2][:, (j4 % 2) * 256 : (j4 % 2) * 256 + 256]
                    for k in range(HT):
                        nc.tensor.matmul(
                            oap, wih_sb[k][:, j * 128 : (j + 1) * 128],
                            xt[:, k * 256 : (k + 1) * 256],
                            start=(k == 0), stop=False)

                def whh_close(ps, j4, j0):
                    j = j0 + j4
                    oap = ps[j4 // 2][:, (j4 % 2) * 256 : (j4 % 2) * 256 + 256]
                    for k in range(HT):
                        nc.tensor.matmul(
                            oap, whh_sb[k][:, j * 128 : (j + 1) * 128],
                            s_cur[:, k * 256 : (k + 1) * 256],
                            start=False, stop=(k == HT - 1))

                def gh_bank(h):
                    for k4 in (2 * h, 2 * h + 1):
                        j = 8 + k4
                        oap = gh_ps[h][:, (k4 % 2) * 256 : (k4 % 2) * 256 + 256]
                        for k in range(HT):
                            nc.tensor.matmul(
                                oap, whh_sb[k][:, j * 128 : (j + 1) * 128],
                                s_cur[:, k * 256 : (k + 1) * 256],
                                start=(k == 0), stop=(k == HT - 1))

                # independent x-projections first (one open per bank),
                # then per-bank sequential close/open/close
                wih_open(r_ps, 0, 0)
                wih_open(r_ps, 2, 0)
                wih_open(z_ps, 0, 4)
                wih_open(z_ps, 2, 4)
                whh_close(r_ps, 0, 0)
                wih_open(r_ps, 1, 0)
                whh_close(r_ps, 1, 0)
                whh_close(r_ps, 2, 0)
                wih_open(r_ps, 3, 0)
                whh_close(r_ps, 3, 0)
                gh_bank(0)
                gh_bank(1)
                whh_close(z_ps, 0, 4)
                wih_open(z_ps, 1, 4)
                whh_close(z_ps, 1, 4)
                whh_close(z_ps, 2, 4)
                wih_open(z_ps, 3, 4)
                whh_close(z_ps, 3, 4)
                if t + 1 < T:
                    gx_nxt = prefill_gx(t + 1, xt_nxt)

                # ---- ACT: r sigmoids (bf16 out) ----
                r_t = sb.tile([128, SF], BF16, name=f"rt{t}", tag="rt", bufs=2)
                for k in range(HT):
                    nc.scalar.activation(
                        out=r_t[:, k * 256 : (k + 1) * 256],
                        in_=r_ps[k // 2][:, (k % 2) * 256 : (k % 2) * 256 + 256],
                        func=AF.Sigmoid, bias=brz_sb[:, k : k + 1], scale=1.0)

                # ---- DVE: w = (gh + bhn) * r  (per k-tile) ----
                w_t = sb.tile([128, SF], BF16, name=f"wt{t}", tag="wt", bufs=2)
                for k in range(HT):
                    nc.vector.scalar_tensor_tensor(
                        out=w_t[:, k * 256 : (k + 1) * 256],
                        in0=gh_ps[k // 2][:, (k % 2) * 256 : (k % 2) * 256 + 256],
                        scalar=bhn_sb[:, k : k + 1],
                        in1=r_t[:, k * 256 : (k + 1) * 256],
                        op0=ALU.add, op1=ALU.mult)

                # ---- DVE: u = w + gx (per X bank, [128,512]);
                #      Pool cannot read PSUM on TRN2 ----
                u_t = sb.tile([128, SF], BF16, name=f"ut{t}", tag="ut", bufs=2)
                for h in range(2):
                    nc.vector.tensor_add(
                        u_t[:, h * 512 : (h + 1) * 512],
                        w_t[:, h * 512 : (h + 1) * 512], gx_ps[h])

                # ---- ACT: tanh (per k-tile, bias=b_ih_n) / z sigmoids ----
                n_t = sb.tile([128, SF], BF16, name=f"nt{t}", tag="nt", bufs=2)
                z_t = sb.tile([128, SF], BF16, name=f"zt{t}", tag="zt", bufs=2)

                def tanh_k(k):
                    nc.scalar.activation(
                        out=n_t[:, k * 256 : (k + 1) * 256],
                        in_=u_t[:, k * 256 : (k + 1) * 256],
                        func=AF.Tanh, bias=bin_sb[:, k : k + 1], scale=1.0)

                def zsig_k(k):
                    nc.scalar.activation(
                        out=z_t[:, k * 256 : (k + 1) * 256],
                        in_=z_ps[k // 2][:, (k % 2) * 256 : (k % 2) * 256 + 256],
                        func=AF.Sigmoid, bias=brz_sb[:, 4 + k : 5 + k], scale=1.0)

                tanh_k(0)
                tanh_k(1)
                zsig_k(0)
                zsig_k(1)
                tanh_k(2)
                tanh_k(3)
                zsig_k(2)
                zsig_k(3)

                # ---- DVE chain (two [128,512] halves):
                #      t1 = s - n; t1 *= z; hn = t1 + n; s' = hn*mk;
                #      y = hn + x; y2 = y*y ----
                hn = sb.tile([128, SF], BF16, name=f"hn{t}", tag="hn", bufs=2)
                t1 = sb.tile([128, SF], BF16, name=f"t1{t}", tag="t1", bufs=2)
                s_nxt = None
                if t + 1 < T:
                    s_nxt = sb.tile([128, SF], BF16, name=f"s{t + 1}",
                                    tag="state", bufs=3)
                if main and toff == 0:
                    y_all = sb.tile([128, HT * FB], BF16,
                                    name=f"y{blk}", tag="y_all", bufs=2)
                    y2_all = sb.tile([128, HT * FB], BF16,
                                     name=f"y2{blk}", tag="y2_all", bufs=1)

                for h in range(2):
                    sl = slice(h * 512, (h + 1) * 512)
                    nc.vector.tensor_sub(t1[:, sl], s_cur[:, sl], n_t[:, sl])
                    nc.vector.tensor_mul(t1[:, sl], t1[:, sl], z_t[:, sl])
                    nc.vector.tensor_add(hn[:, sl], t1[:, sl], n_t[:, sl])
                    if s_nxt is not None:
                        for k in (2 * h, 2 * h + 1):
                            ksl = slice(k * 256, (k + 1) * 256)
                            nc.vector.tensor_mul(
                                s_nxt[:, ksl], hn[:, ksl], mk)
                if main:
                    for k in range(HT):
                        o = k * FB + toff * S
                        ksl = slice(k * 256, (k + 1) * 256)
                        nc.gpsimd.tensor_add(
                            y_all[:, o : o + S], hn[:, ksl], xt[:, ksl])
                        nc.gpsimd.tensor_mul(
                            y2_all[:, o : o + S],
                            y_all[:, o : o + S], y_all[:, o : o + S])

                if debug and t == 0:
                    pass
                    nc.sync.dma_start(out=dbg_d["r0"][:, :], in_=r_t)
                    nc.sync.dma_start(out=dbg_d["z0"][:, :], in_=z_t)
                    nc.sync.dma_start(out=dbg_d["n0"][:, :], in_=n_t)
                    nc.sync.dma_start(out=dbg_d["hn0"][:, :], in_=hn)
                    nc.sync.dma_start(out=dbg_d["s1"][:, :], in_=s_nxt)
                if debug and t == 1:
                    nc.sync.dma_start(out=dbg_d["s2x"][:, :], in_=s_nxt)
                if debug and t == T - 1:
                    nc.sync.dma_start(out=dbg_d["hnT"][:, :], in_=hn)

                # -- h0 injection at entry to main (chunk-0 columns) --
                if t + 1 == R:
                    inj = s_nxt.rearrange("p (k c) -> p k c", k=HT)[:, :, 0:S:C]
                    nc.vector.tensor_copy(
                        inj, h0m_sb.rearrange("p (k n) -> p k n", k=HT))

                # -- final hidden state (chunk C-1 columns); compact on
                #    DVE first (a strided DMA lowers to 2-byte packets) --
                if t == T - 1:
                    hl = hn.rearrange("p (k c) -> p k c", k=HT)[
                        :, :, C - 1 : S : C]
                    hlc = sb.tile([128, HT * NB], BF16, name="hlc", tag="hlc")
                    nc.vector.tensor_copy(
                        hlc.rearrange("p (k n) -> p k n", k=HT), hl)
                    nc.sync.dma_start(out=hl_d[:, :], in_=hlc)

                # ---- s2 (ACT tail): sqrt (costs 2 act-table loads) ----
                for dt_, b, st in stages:
                    if dt_ == 2:
                        sq = sb.tile([4, 512], F32, name=f"sq{b}", tag="sq",
                                     bufs=2)
                        nc.scalar.activation(
                            out=sq, in_=st["dvar"], func=AF.Sqrt,
                            bias=eps_sb[0:4, :],
                            scale=1.0 / (float(H) * float(H)))
                        st["sq"] = sq

                # ---- yn stages (DVE tail): normalize one k-tile ----
                for dt_, b, st in stages:
                    if 5 <= dt_ <= 8:
                        k = dt_ - 5
                        yn = sb.tile([128, FB], BF16, name=f"yn{b}_{k}",
                                     tag="yn", bufs=2)
                        nc.gpsimd.tensor_mul(
                            yn, st["y_all"][:, k * FB : (k + 1) * FB],
                            st["rs_bc"])
                        nc.vector.tensor_sub(yn, yn, st["mu_bc"])
                        if not triv_gb:
                            nc.gpsimd.tensor_scalar(
                                out=yn, in0=yn,
                                scalar1=gam_sb[:, k : k + 1],
                                scalar2=bet_sb[:, k : k + 1],
                                op0=ALU.mult, op1=ALU.add)
                        nc.sync.dma_start(
                            out=out_d[k, :, b * FB : (b + 1) * FB], in_=yn)
                        if k == HT - 1:
                            del ln[b]

                # ---- bc waves (PE tail): broadcast Smu/rs rows to
                #      [128,512] PSUM tiles via selector matmuls, borrowing
                #      the gh and z bank pairs (complete 1-mm groups) ----
                for dt_, b, st in stages:
                    if dt_ in (3, 4):
                        waves = (0, 1) if dt_ == 3 else (2, 3)
                        for i, g in enumerate(waves):
                            tg = ("gh0", "gh1") if i == 0 else ("r0", "r1")
                            mu_ps = rzp.tile([128, 512], F32,
                                             name=f"bcm{b}_{g}", tag=tg[0])
                            rs_ps = rzp.tile([128, 512], F32,
                                             name=f"bcr{b}_{g}", tag=tg[1])
                            nc.tensor.matmul(
                                mu_ps, sel_sb[0:4, g * 128 : (g + 1) * 128],
                                st["pkm"], start=True, stop=True,
                                skip_group_check=True)
                            nc.tensor.matmul(
                                rs_ps, sel_sb[0:4, g * 128 : (g + 1) * 128],
                                st["pkr"], start=True, stop=True,
                                skip_group_check=True)
                            nc.scalar.activation(
                                out=st["mu_bc"][:, g * 512 : (g + 1) * 512],
                                in_=mu_ps, func=AF.Identity, scale=1.0)
                            nc.scalar.activation(
                                out=st["rs_bc"][:, g * 512 : (g + 1) * 512],
                                in_=rs_ps, func=AF.Identity, scale=1.0)

                # ---- blk_end (PE tail): LN column-sum matmuls ----
                if blk_end:
                    stp0 = rzp.tile([128, 512], F32, name=f"st{blk}a",
                                    tag="gh0")
                    stp1 = rzp.tile([128, 512], F32, name=f"st{blk}b",
                                    tag="gh1")
                    for g in range(4):   # Smu rows 0..3 <- y cols g*512..
                        for k in range(HT):
                            nc.tensor.matmul(
                                stp0[0:4, :],
                                ind_sb[:, g * 4 : (g + 1) * 4],
                                y_all[:, k * FB + g * 512 : k * FB + g * 512 + 512],
                                start=(g == 0 and k == 0),
                                stop=(g == 3 and k == HT - 1),
                                skip_group_check=True)
                    for g in range(4):   # Sss rows 0..3 <- y2
                        for k in range(HT):
                            nc.tensor.matmul(
                                stp1[0:4, :],
                                ind_sb[:, g * 4 : (g + 1) * 4],
                                y2_all[:, k * FB + g * 512 : k * FB + g * 512 + 512],
                                start=(g == 0 and k == 0),
                                stop=(g == 3 and k == HT - 1),
                                skip_group_check=True)
                    ln[blk] = {"te": t, "stp0": stp0, "stp1": stp1,
                               "y_all": y_all}

                if t + 1 < T:
                    s_cur = s_nxt
                    xt = xt_nxt
                    gx_ps = gx_nxt

            # ---- tail: finish LN for the last block(s) ----
            for b in sorted(ln):
                st = ln[b]
                st_mu = sb.tile([4, 512], F32, name=f"tstm{b}", tag="stm",
                                bufs=2)
                nc.scalar.activation(
                    out=st_mu, in_=st["stp0"][0:4, :], func=AF.Identity,
                    scale=1.0)
                st_ss = sb.tile([4, 512], F32, name=f"tstv{b}", tag="stv",
                                bufs=2)
                nc.scalar.activation(
                    out=st_ss, in_=st["stp1"][0:4, :], func=AF.Identity,
                    scale=1.0)
                musq = sb.tile([4, 512], F32, name=f"tmq{b}", tag="mq", bufs=2)
                nc.vector.tensor_mul(musq, st_mu, st_mu)
                dvar = sb.tile([4, 512], F32, name=f"tdv{b}", tag="dv", bufs=2)
                nc.vector.scalar_tensor_tensor(
                    out=dvar, in0=st_ss, scalar=float(H), in1=musq,
                    op0=ALU.mult, op1=ALU.subtract)
                sq = sb.tile([4, 512], F32, name=f"tsq{b}", tag="sq", bufs=2)
                nc.scalar.activation(
                    out=sq, in_=dvar, func=AF.Sqrt,
                    bias=eps_sb[0:4, :],
                    scale=1.0 / (float(H) * float(H)))
                rst = sb.tile([4, 512], F32, name=f"trs{b}", tag="rs", bufs=2)
                nc.vector.reciprocal_approx_fast(out=rst, in_=sq)
                bsm = sb.tile([4, 512], F32, name=f"tbs{b}", tag="bs", bufs=2)
                nc.vector.scalar_tensor_tensor(
                    out=bsm, in0=st_mu, scalar=1.0 / float(H),
                    in1=rst, op0=ALU.mult, op1=ALU.mult)
                pkm = sb.tile([4, 512], BF16, name=f"tpkm{b}", tag="pkm",
                              bufs=2)
                nc.vector.tensor_copy(pkm, bsm)
                pkr = sb.tile([4, 512], BF16, name=f"tpkr{b}", tag="pkr",
                              bufs=2)
                nc.vector.tensor_copy(pkr, rst)
                mu_bc = sb.tile([128, FB], BF16, name=f"tmubc{b}", tag="mubc",
                                bufs=1)
                rs_bc = sb.tile([128, FB], BF16, name=f"trsbc{b}", tag="rsbc",
                                bufs=1)
                tags = [("gh0", "gh1"), ("z0", "z1"), ("r0", "r1"),
                        ("gx0", "gx1")]
                for g in range(4):
                    tg = tags[g]
                    pool = gxp if tg[0].startswith("gx") else rzp
                    mu_ps = rzp.tile([128, 512], F32, name=f"tbm{b}_{g}",
                                     tag=tg[0]) if tg[0] != "gx0" else                         gxp.tile([128, 512], F32, name=f"tbm{b}_{g}",
                                 tag=tg[0])
                    rs_ps = rzp.tile([128, 512], F32, name=f"tbr{b}_{g}",
                                     tag=tg[1]) if tg[1] != "gx1" else                         gxp.tile([128, 512], F32, name=f"tbr{b}_{g}",
                                 tag=tg[1])
                    nc.tensor.matmul(
                        mu_ps, sel_sb[0:4, g * 128 : (g + 1) * 128], pkm,
                        start=True, stop=True, skip_group_check=True)
                    nc.tensor.matmul(
                        rs_ps, sel_sb[0:4, g * 128 : (g + 1) * 128], pkr,
                        start=True, stop=True, skip_group_check=True)
                    nc.scalar.activation(
                        out=mu_bc[:, g * 512 : (g + 1) * 512], in_=mu_ps,
                        func=AF.Identity, scale=1.0)
                    nc.scalar.activation(
                        out=rs_bc[:, g * 512 : (g + 1) * 512], in_=rs_ps,
                        func=AF.Identity, scale=1.0)
                for k in range(HT):
                    yn = sb.tile([128, FB], BF16, name=f"tyn{b}_{k}",
                                 tag="yn", bufs=2)
                    nc.gpsimd.tensor_mul(
                        yn, st["y_all"][:, k * FB : (k + 1) * FB], rs_bc)
                    nc.vector.tensor_sub(yn, yn, mu_bc)
                    if not triv_gb:
                        nc.gpsimd.tensor_scalar(
                            out=yn, in0=yn,
                            scalar1=gam_sb[:, k : k + 1],
                            scalar2=bet_sb[:, k : k + 1],
                            op0=ALU.mult, op1=ALU.add)
                    nc.sync.dma_start(
                        out=out_d[k, :, b * FB : (b + 1) * FB], in_=yn)
    nc.compile()
    return nc


def stage_inputs(input, h, is_initial, W_ih, W_hh, b_ih, b_hh, gamma, beta, R):
    """Host-side sharding/staging. Returns per-core input maps."""
    import ml_dtypes

    T = R + KS
    x = np.asarray(input, np.float32)
    h0 = np.asarray(h, np.float32)
    ii = np.asarray(is_initial).reshape(N, L)
    W_ih = np.asarray(W_ih, np.float32)
    W_hh = np.asarray(W_hh, np.float32)
    b_ih = np.asarray(b_ih, np.float32)
    b_hh = np.asarray(b_hh, np.float32)
    gamma = np.asarray(gamma, np.float32)
    beta = np.asarray(beta, np.float32)

    def bf(a):
        return np.ascontiguousarray(np.asarray(a, np.float32)).astype(
            ml_dtypes.bfloat16)

    mask = 1.0 - ii.astype(np.float32)  # [N, L]

    # l index per (c, t): warm-up reads the R steps before the chunk;
    # chunk 0's warm-up reads l in [KS-R, KS) (discarded garbage).
    l_for = np.empty((C, T), np.int64)
    for c in range(C):
        for t in range(T):
            l = c * KS + (t - R)
            l_for[c, t] = l if l >= 0 else l + KS

    wihT = np.ascontiguousarray(W_ih.T.reshape(HT, 128, 3 * H))
    whhT = np.ascontiguousarray(W_hh.T.reshape(HT, 128, 3 * H))
    brz = (b_ih + b_hh)[: 2 * H].reshape(8, 128).T.copy()        # [128, 8]
    bhn = b_hh[2 * H :].reshape(HT, 128).T.copy()                # [128, 4]
    binn = b_ih[2 * H :].reshape(HT, 128).T.copy()
    gam = gamma.reshape(HT, 128).T.copy()
    bet = beta.reshape(HT, 128).T.copy()
    # indicator stationary: group g (of 4) is a [128, 4] tile whose
    # column g is all-ones (routes a column-sum into PSUM partition g)
    ind = np.zeros((128, 16), np.float32)
    for g in range(4):
        ind[:, g * 4 + g] = 1.0
    # selector for the PSUM->all-partitions broadcast matmuls:
    # sel[c, g*128+po] = 1 iff c == g
    sel = np.zeros((4, 512), np.float32)
    for g in range(4):
        sel[g, g * 128 : (g + 1) * 128] = 1.0

    in_maps = []
    for core in range(NCORES):
        n0 = core * NB
        xc = x[n0 : n0 + NB]              # [NB, L, H]
        xg = xc[:, l_for, :]              # [NB, C, T, H]
        # xs2[t, p, k*S + s] with s = n*C + c, h = k*128 + p
        xs2 = np.ascontiguousarray(
            xg.transpose(2, 3, 0, 1)      # [T, H, NB, C]
            .reshape(T, HT, 128, S)
            .transpose(0, 2, 1, 3)        # [T, 128, HT, S]
            .reshape(T, 128, SF))
        mg = mask[n0 : n0 + NB][:, l_for]  # [NB, C, T]
        ms = np.ascontiguousarray(mg.transpose(2, 0, 1).reshape(T, S))
        m0 = mask[n0 : n0 + NB, 0]         # [NB]
        h0c = h0[n0 : n0 + NB] * m0[:, None]     # [NB, H]
        # h0m[p, k*NB + n] = h0c[n, k*128+p]
        h0m = np.ascontiguousarray(
            h0c.reshape(NB, HT, 128).transpose(2, 1, 0).reshape(128, HT * NB))
        in_maps.append({
            "xs": bf(xs2), "ms": bf(ms), "h0m": bf(h0m),
            "wih": bf(wihT), "whh": bf(whhT),
            "brz": brz, "bhn": bhn, "bin": binn,
            "gam": gam, "bet": bet, "ind": bf(ind), "sel": bf(sel),
        })
    return in_maps


def required_warmup(is_initial):
    """Max distance from a chunk boundary back to the latest reset."""
    ii = np.asarray(is_initial).reshape(N, L)
    need = 0
    for c in range(1, C):
        start = c * KS
        sub = ii[:, :start]
        for n in range(N):
            nz = np.nonzero(sub[n])[0]
            gap = start - nz[-1] if len(nz) else start
            need = max(need, gap)
    return need


def unstage_outputs(results):
    out = np.empty((N, L, H), np.float32)
    h_last = np.empty((N, H), np.float32)
    for core in range(NCORES):
        n0 = core * NB
        st = np.asarray(results[core]["out_st"], np.float32)  # [HT,128,KS*S]
        o = st.reshape(HT, 128, KS, NB, C).transpose(3, 4, 2, 0, 1)
        out[n0 : n0 + NB] = o.reshape(NB, L, H)
        hl = np.asarray(results[core]["hlast"], np.float32)   # [128, HT*NB]
        h_last[n0 : n0 + NB] = (
            hl.reshape(128, HT, NB).transpose(2, 1, 0).reshape(NB, H))
    h_exp = np.broadcast_to(h_last[:, None, :], (N, L, H)).copy()
    return out, h_exp


_PROGRAM_CACHE = {}


def kernel(input, h, is_initial, W_ih, W_hh, b_ih, b_hh, gamma, beta):
    need = required_warmup(is_initial)
    R = max(12, int(need))
    triv = bool(
        np.all(np.asarray(gamma) == 1.0) and np.all(np.asarray(beta) == 0.0))
    key = (R, triv)
    if key not in _PROGRAM_CACHE:
        _PROGRAM_CACHE[key] = build_program(R, triv_gb=triv)
    nc = _PROGRAM_CACHE[key]
    in_maps = stage_inputs(
        input, h, is_initial, W_ih, W_hh, b_ih, b_hh, gamma, beta, R)
    res = run_bass_kernel_spmd(nc, in_maps, list(range(NCORES))).results
    return unstage_outputs(res)


# revision 37
# speedup vs baseline: 1.1853x; 1.1853x over previous
"""Trainium2 Bass kernel for masked-GRU + residual + LayerNorm.

Problem: N=128 sequences of length L=512, hidden H=512.
  gx = x @ W_ih.T + b_ih            (precomputable input projection)
  per step l: hc = h * (1-is_initial[l]); gh = hc @ W_hh.T + b_hh
    r = sig(gx_r+gh_r); z = sig(gx_z+gh_z); n = tanh(gx_n + r*gh_n)
    h = (1-z)*n + z*hc
  out = LayerNorm(seq + x) * gamma + beta;  h_exp = broadcast(h_last)

Strategy (v2):
  * Data parallel: 16 batch rows per core (8 cores).
  * Sequence-chunk parallel: each L=512 sequence is split into C=16
    chunks of 32 steps, processed as independent columns, made exact by
    an R-step warm-up (a reset lands inside every R-window; verified at
    runtime). Chunk 0 injects the true h0 at entry to main.
  * bf16 everywhere off-PSUM: weights, x, gates, state, y, outputs.
    PSUM stays f32.  DVE runs 2-byte all-SBUF ops at 4x rate.
  * Packed state s [128, 1024] bf16 (4 h-tiles side by side in the free
    dim) so the elementwise chain runs as [128,512] halves.
  * PSUM banks (8): R pair (r gates), Z pair (z), G pair (gh_n),
    X pair (gx_n, lives across one step boundary).  LN stats borrow
    bank G0 right after it drains.
  * n-gate: gx_n stays in PSUM; w=(gh+bhh_n)*r on DVE, u=w+gx on Pool,
    tanh(u)+b_ih_n on ACT.  No identity-drain of gx.
  * LayerNorm per block of 8 steps, software-pipelined over the 7
    following steps so nothing blocks the recurrence: column sums via
    indicator-stationary matmuls into ONE [8,512] PSUM tile (rows 0-3
    Smu, rows 4-7 Sss), stats math as [4,512] multi-partition ops
    (D = H*Sss - Smu^2; rs = 1/sqrt(D + eps*H^2); out = ((H*y - Smu)
    * rs) * gamma + beta), DRAM-bounce broadcast of (Smu, rs) in bf16.
  * out / h_last stored bf16, upcast host-side.
"""
import sys

sys.path.insert(0, "/opt/trn_rl_repo")

import numpy as np

import concourse.bass as bass
import concourse.tile as tile
from concourse import bacc, mybir
from concourse.bass_utils import run_bass_kernel_spmd

F32 = mybir.dt.float32
BF16 = mybir.dt.bfloat16
AF = mybir.ActivationFunctionType
ALU = mybir.AluOpType

N, L, H = 128, 512, 512
NCORES = 8
NB = N // NCORES          # batch rows per core = 16
C = 16                    # chunks per sequence
KS = L // C               # main steps per chunk = 32
S = NB * C                # columns per core = 256
HT = H // 128             # h partition tiles = 4
BLK = 8                   # LN block (main steps)
NBLK = KS // BLK          # 4
FB = BLK * S              # 2048 block columns
SF = HT * S               # 1024 free dim of the packed state


def _bcast_ap(row_ap, parts=128, rep=1):
    """DRAM row AP -> partition-broadcast AP (0-stride over partitions,
    optionally replicated rep times along free)."""
    ap = [[0, parts]]
    if rep > 1:
        ap.append([0, rep])
    ap += [list(d) for d in row_ap.ap]
    return bass.AP(tensor=row_ap.tensor, offset=row_ap.offset, ap=ap)


def build_program(R=12, debug=False, triv_gb=True):
    T = R + KS
    nc = bacc.Bacc("TRN2", target_bir_lowering=False)

    xs_d = nc.declare_dram_parameter("xs", [T, 128, SF], BF16, isOutput=False)
    ms_d = nc.declare_dram_parameter("ms", [T, S], BF16, isOutput=False)
    h0m_d = nc.declare_dram_parameter("h0m", [128, HT * NB], BF16, isOutput=False)
    wih_d = nc.declare_dram_parameter("wih", [HT, 128, 3 * H], BF16, isOutput=False)
    whh_d = nc.declare_dram_parameter("whh", [HT, 128, 3 * H], BF16, isOutput=False)
    brz_d = nc.declare_dram_parameter("brz", [128, 8], F32, isOutput=False)
    bhn_d = nc.declare_dram_parameter("bhn", [128, HT], F32, isOutput=False)
    bin_d = nc.declare_dram_parameter("bin", [128, HT], F32, isOutput=False)
    gam_d = nc.declare_dram_parameter("gam", [128, HT], F32, isOutput=False)
    bet_d = nc.declare_dram_parameter("bet", [128, HT], F32, isOutput=False)
    ind_d = nc.declare_dram_parameter("ind", [128, 16], BF16, isOutput=False)
    sel_d = nc.declare_dram_parameter("sel", [4, 512], BF16, isOutput=False)

    out_d = nc.declare_dram_parameter("out_st", [HT, 128, KS * S], BF16, isOutput=True)
    if debug:
        dbg_names = ["mk0", "r0", "z0", "n0", "hn0", "s1", "s2x", "hnT"]
        dbg_d = {
            nm: nc.declare_dram_parameter(f"dbg_{nm}", [128, SF], BF16,
                                          isOutput=True)
            for nm in dbg_names
        }
    hl_d = nc.declare_dram_parameter("hlast", [128, HT * NB], BF16, isOutput=True)


    with tile.TileContext(nc) as tc:
        with (
            tc.tile_pool(name="const", bufs=1) as cst,
            tc.tile_pool(name="sb", bufs=1) as sb,
            tc.tile_pool(name="rzp", bufs=1, space="PSUM") as rzp,
            tc.tile_pool(name="gxp", bufs=1, space="PSUM") as gxp,
        ):
            # ---- constants (wih first: needed by the t=0 prefill) ----
            x0 = sb.tile([128, SF], BF16, name="xt0", tag="xt", bufs=3)
            nc.sync.dma_start(out=x0, in_=xs_d[0, :, :])
            wih_sb, whh_sb = [], []
            for k in range(HT):
                w1 = cst.tile([128, 3 * H], BF16, name=f"wih_sb{k}", tag=f"wih{k}")
                (nc.sync if k % 2 else nc.scalar).dma_start(
                    out=w1, in_=wih_d[k, :, :])
                wih_sb.append(w1)
            for k in range(HT):
                w2 = cst.tile([128, 3 * H], BF16, name=f"whh_sb{k}", tag=f"whh{k}")
                (nc.sync if k % 2 else nc.scalar).dma_start(
                    out=w2, in_=whh_d[k, :, :])
                whh_sb.append(w2)
            h0m_sb = cst.tile([128, HT * NB], BF16, name="h0m_sb", tag="h0m")
            nc.sync.dma_start(out=h0m_sb, in_=h0m_d[:, :])
            brz_sb = cst.tile([128, 8], F32, name="brz_sb", tag="brz")
            nc.sync.dma_start(out=brz_sb, in_=brz_d[:, :])
            bhn_sb = cst.tile([128, HT], F32, name="bhn_sb", tag="bhn")
            nc.sync.dma_start(out=bhn_sb, in_=bhn_d[:, :])
            bin_sb = cst.tile([128, HT], F32, name="bin_sb", tag="bin")
            nc.sync.dma_start(out=bin_sb, in_=bin_d[:, :])
            gam_sb = cst.tile([128, HT], F32, name="gam_sb", tag="gam")
            nc.sync.dma_start(out=gam_sb, in_=gam_d[:, :])
            bet_sb = cst.tile([128, HT], F32, name="bet_sb", tag="bet")
            nc.sync.dma_start(out=bet_sb, in_=bet_d[:, :])
            ind_sb = cst.tile([128, 16], BF16, name="ind_sb", tag="ind")
            nc.sync.dma_start(out=ind_sb, in_=ind_d[:, :])
            eps_sb = cst.tile([128, 1], F32, name="eps_sb", tag="eps")
            nc.vector.memset(eps_sb, 1e-5)
            sel_sb = cst.tile([4, 512], BF16, name="sel_sb", tag="sel")
            nc.sync.dma_start(out=sel_sb, in_=sel_d[:, :])
            # masks: load once to partition 0, broadcast on Pool
            ms_all = cst.tile([128, T * S], BF16, name="ms_all", tag="msb")
            nc.sync.dma_start(
                out=ms_all[0:1, :], in_=ms_d[:, :].rearrange("t s -> (t s)"))
            nc.gpsimd.partition_broadcast(ms_all, ms_all[0:1, :])

            # ---- initial (zero) state ----
            s_cur = sb.tile([128, SF], BF16, name="s_init", tag="state", bufs=3)
            nc.vector.memset(s_cur, 0.0)

            def load_x(t):
                xt = sb.tile([128, SF], BF16, name=f"xt{t}", tag="xt", bufs=3)
                nc.sync.dma_start(out=xt, in_=xs_d[t, :, :])
                return xt



            def prefill_gx(t, xt):
                # open+close gx_n groups for step t (wih only); X pair
                # [gx8|gx9], [gx10|gx11]; stays in PSUM until step t's u.
                gx_ps = [
                    gxp.tile([128, 512], F32, name=f"gx{t}_{j}", tag=f"gx{j}")
                    for j in range(2)
                ]
                for k4 in range(4):
                    j = 8 + k4
                    oap = gx_ps[k4 // 2][:, (k4 % 2) * 256 : (k4 % 2) * 256 + 256]
                    for k in range(HT):
                        nc.tensor.matmul(
                            oap, wih_sb[k][:, j * 128 : (j + 1) * 128],
                            xt[:, k * 256 : (k + 1) * 256],
                            start=(k == 0), stop=(k == HT - 1))
                return gx_ps

            xt = x0
            gx_ps = prefill_gx(0, xt)

            # LN pipeline state, keyed by block id
            ln = {}
            y_all = y2_all = None

            for t in range(T):
                main = t >= R
                toff = (t - R) % BLK
                blk = (t - R) // BLK
                blk_end = main and toff == BLK - 1

                # LN pipeline stage for earlier blocks this iteration
                stages = [(t - st["te"], b, st) for b, st in list(ln.items())
                          if 1 <= t - st["te"] <= 8]

                # ---- s1 (ACT top): drain the stats PSUM banks ----
                for dt_, b, st in stages:
                    if dt_ == 1:
                        st_mu = sb.tile([4, 512], F32, name=f"stm{b}",
                                        tag="stm", bufs=2)
                        nc.scalar.activation(
                            out=st_mu, in_=st["stp0"][0:4, :],
                            func=AF.Identity, scale=1.0)
                        st_ss = sb.tile([4, 512], F32, name=f"stv{b}",
                                        tag="stv", bufs=2)
                        nc.scalar.activation(
                            out=st_ss, in_=st["stp1"][0:4, :],
                            func=AF.Identity, scale=1.0)
                        st["st_mu"] = st_mu
                        st["st_ss"] = st_ss

                if t + 1 < T:
                    xt_nxt = load_x(t + 1)
                    mk = ms_all[:, (t + 1) * S : (t + 2) * S]

                # ---- s1 (DVE top): musq, dvar ----
                for dt_, b, st in stages:
                    if dt_ == 1:
                        musq = sb.tile([4, 512], F32, name=f"mq{b}", tag="mq",
                                       bufs=2)
                        nc.vector.tensor_mul(
                            musq, st["st_mu"], st["st_mu"])
                        dvar = sb.tile([4, 512], F32, name=f"dv{b}", tag="dv",
                                       bufs=2)
                        nc.vector.scalar_tensor_tensor(
                            out=dvar, in0=st["st_ss"],
                            scalar=float(H), in1=musq,
                            op0=ALU.mult, op1=ALU.subtract)
                        st["dvar"] = dvar

                # ---- s3 (DVE top): recip + bf16 packs ----
                for dt_, b, st in stages:
                    if dt_ == 3:
                        rst = sb.tile([4, 512], F32, name=f"rs{b}", tag="rs",
                                      bufs=2)
                        nc.vector.reciprocal_approx_fast(out=rst, in_=st["sq"])
                        bsm = sb.tile([4, 512], F32, name=f"bs{b}", tag="bs",
                                      bufs=2)
                        nc.vector.scalar_tensor_tensor(
                            out=bsm, in0=st["st_mu"], scalar=1.0 / float(H),
                            in1=rst, op0=ALU.mult, op1=ALU.mult)
                        pkm = sb.tile([4, 512], BF16, name=f"pkm{b}",
                                      tag="pkm", bufs=2)
                        nc.vector.tensor_copy(pkm, bsm)
                        pkr = sb.tile([4, 512], BF16, name=f"pkr{b}",
                                      tag="pkr", bufs=2)
                        nc.vector.tensor_copy(pkr, rst)
                        st["pkm"] = pkm
                        st["pkr"] = pkr
                        st["mu_bc"] = sb.tile([128, FB], BF16, name=f"mubc{b}",
                                              tag="mubc", bufs=1)
                        st["rs_bc"] = sb.tile([128, FB], BF16, name=f"rsbc{b}",
                                              tag="rsbc", bufs=1)



                # ---- PE: all r/z/gh groups open and close within this
                #      iteration (cross-iteration open groups on sliced
                #      tiles miscompile).  Order: independent wih opens
                #      first, state-dependent whh closes mid-stream, the
                #      complete gx prefill for t+1 last. ----
                r_ps = [
                    rzp.tile([128, 512], F32, name=f"r{t}_{j}", tag=f"r{j}")
                    for j in range(2)
                ]
                gh_ps = [
                    rzp.tile([128, 512], F32, name=f"gh{t}_{j}", tag=f"gh{j}")
                    for j in range(2)
                ]
                z_ps = [
                    rzp.tile([128, 512], F32, name=f"z{t}_{j}", tag=f"z{j}")
                    for j in range(2)
                ]

                def wih_open(ps, j4, j0):
                    # opens the j4 slice group (start zeroes the bank's
                    # write-bitmap: no other start may hit this bank until
                    # this group fully closes)
                    j = j0 + j4
                    oap = ps[j4 // 2][:, (j4 % 2) * 256 : (j4 % 2) * 256 + 256]
                    for k in range(HT):
                        nc.tensor.matmul(
                            oap, wih_sb[k][:, j * 128 : (j + 1) * 128],
                            xt[:, k * 256 : (k + 1) * 256],
                            start=(k == 0), stop=False)

                def whh_close(ps, j4, j0):
                    j = j0 + j4
                    oap = ps[j4 // 2][:, (j4 % 2) * 256 : (j4 % 2) * 256 + 256]
                    for k in range(HT):
                        nc.tensor.matmul(
                            oap, whh_sb[k][:, j * 128 : (j + 1) * 128],
                            s_cur[:, k * 256 : (k + 1) * 256],
                            start=False, stop=(k == HT - 1))

                def gh_bank(h):
                    for k4 in (2 * h, 2 * h + 1):
                        j = 8 + k4
                        oap = gh_ps[h][:, (k4 % 2) * 256 : (k4 % 2) * 256 + 256]
                        for k in range(HT):
                            nc.tensor.matmul(
                                oap, whh_sb[k][:, j * 128 : (j + 1) * 128],
                                s_cur[:, k * 256 : (k + 1) * 256],
                                start=(k == 0), stop=(k == HT - 1))

                # independent x-projections first (one open per bank),
                # then per-bank sequential close/open/close
                wih_open(r_ps, 0, 0)
                wih_open(r_ps, 2, 0)
                wih_open(z_ps, 0, 4)
                wih_open(z_ps, 2, 4)
                whh_close(r_ps, 0, 0)
                wih_open(r_ps, 1, 0)
                whh_close(r_ps, 1, 0)
                whh_close(r_ps, 2, 0)
                wih_open(r_ps, 3, 0)
                whh_close(r_ps, 3, 0)
                gh_bank(0)
                gh_bank(1)
                whh_close(z_ps, 0, 4)
                wih_open(z_ps, 1, 4)
                whh_close(z_ps, 1, 4)
                whh_close(z_ps, 2, 4)
                wih_open(z_ps, 3, 4)
                whh_close(z_ps, 3, 4)
                if t + 1 < T:
                    gx_nxt = prefill_gx(t + 1, xt_nxt)

                # ---- ACT: r sigmoids (bf16 out) ----
                r_t = sb.tile([128, SF], BF16, name=f"rt{t}", tag="rt", bufs=2)
                for k in range(HT):
                    nc.scalar.activation(
                        out=r_t[:, k * 256 : (k + 1) * 256],
                        in_=r_ps[k // 2][:, (k % 2) * 256 : (k % 2) * 256 + 256],
                        func=AF.Sigmoid, bias=brz_sb[:, k : k + 1], scale=1.0)

                # ---- DVE: w = (gh + bhn) * r  (per k-tile) ----
                w_t = sb.tile([128, SF], BF16, name=f"wt{t}", tag="wt", bufs=2)
                for k in range(HT):
                    nc.vector.scalar_tensor_tensor(
                        out=w_t[:, k * 256 : (k + 1) * 256],
                        in0=gh_ps[k // 2][:, (k % 2) * 256 : (k % 2) * 256 + 256],
                        scalar=bhn_sb[:, k : k + 1],
                        in1=r_t[:, k * 256 : (k + 1) * 256],
                        op0=ALU.add, op1=ALU.mult)

                # ---- DVE: u = w + gx (per X bank, [128,512]);
                #      Pool cannot read PSUM on TRN2 ----
                u_t = sb.tile([128, SF], BF16, name=f"ut{t}", tag="ut", bufs=2)
                for h in range(2):
                    nc.vector.tensor_add(
                        u_t[:, h * 512 : (h + 1) * 512],
                        w_t[:, h * 512 : (h + 1) * 512], gx_ps[h])

                # ---- ACT: tanh (per k-tile, bias=b_ih_n) / z sigmoids ----
                n_t = sb.tile([128, SF], BF16, name=f"nt{t}", tag="nt", bufs=2)
                z_t = sb.tile([128, SF], BF16, name=f"zt{t}", tag="zt", bufs=2)

                def tanh_k(k):
                    nc.scalar.activation(
                        out=n_t[:, k * 256 : (k + 1) * 256],
                        in_=u_t[:, k * 256 : (k + 1) * 256],
                        func=AF.Tanh, bias=bin_sb[:, k : k + 1], scale=1.0)

                def zsig_k(k):
                    nc.scalar.activation(
                        out=z_t[:, k * 256 : (k + 1) * 256],
                        in_=z_ps[k // 2][:, (k % 2) * 256 : (k % 2) * 256 + 256],
                        func=AF.Sigmoid, bias=brz_sb[:, 4 + k : 5 + k], scale=1.0)

                tanh_k(0)
                tanh_k(1)
                zsig_k(0)
                zsig_k(1)
                tanh_k(2)
                tanh_k(3)
                zsig_k(2)
                zsig_k(3)

                # ---- DVE chain (two [128,512] halves):
                #      t1 = s - n; t1 *= z; hn = t1 + n; s' = hn*mk;
                #      y = hn + x; y2 = y*y ----
                hn = sb.tile([128, SF], BF16, name=f"hn{t}", tag="hn", bufs=2)
                t1 = sb.tile([128, SF], BF16, name=f"t1{t}", tag="t1", bufs=2)
                s_nxt = None
                if t + 1 < T:
                    s_nxt = sb.tile([128, SF], BF16, name=f"s{t + 1}",
                                    tag="state", bufs=3)
                if main and toff == 0:
                    y_all = sb.tile([128, HT * FB], BF16,
                                    name=f"y{blk}", tag="y_all", bufs=2)
                    y2_all = sb.tile([128, HT * FB], BF16,
                                     name=f"y2{blk}", tag="y2_all", bufs=1)

                for h in range(2):
                    sl = slice(h * 512, (h + 1) * 512)
                    nc.vector.tensor_sub(t1[:, sl], s_cur[:, sl], n_t[:, sl])
                    nc.vector.tensor_mul(t1[:, sl], t1[:, sl], z_t[:, sl])
                    nc.vector.tensor_add(hn[:, sl], t1[:, sl], n_t[:, sl])
                    if s_nxt is not None:
                        for k in (2 * h, 2 * h + 1):
                            ksl = slice(k * 256, (k + 1) * 256)
                            nc.vector.tensor_mul(
                                s_nxt[:, ksl], hn[:, ksl], mk)
                if main:
                    for k in range(HT):
                        o = k * FB + toff * S
                        ksl = slice(k * 256, (k + 1) * 256)
                        nc.vector.tensor_add(
                            y_all[:, o : o + S], hn[:, ksl], xt[:, ksl])
                        nc.gpsimd.tensor_mul(
                            y2_all[:, o : o + S],
                            y_all[:, o : o + S], y_all[:, o : o + S])

                if debug and t == 0:
                    pass
                    nc.sync.dma_start(out=dbg_d["r0"][:, :], in_=r_t)
                    nc.sync.dma_start(out=dbg_d["z0"][:, :], in_=z_t)
                    nc.sync.dma_start(out=dbg_d["n0"][:, :], in_=n_t)
                    nc.sync.dma_start(out=dbg_d["hn0"][:, :], in_=hn)
                    nc.sync.dma_start(out=dbg_d["s1"][:, :], in_=s_nxt)
                if debug and t == 1:
                    nc.sync.dma_start(out=dbg_d["s2x"][:, :], in_=s_nxt)
                if debug and t == T - 1:
                    nc.sync.dma_start(out=dbg_d["hnT"][:, :], in_=hn)

                # -- h0 injection at entry to main (chunk-0 columns) --
                if t + 1 == R:
                    inj = s_nxt.rearrange("p (k c) -> p k c", k=HT)[:, :, 0:S:C]
                    nc.vector.tensor_copy(
                        inj, h0m_sb.rearrange("p (k n) -> p k n", k=HT))

                # -- final hidden state (chunk C-1 columns); compact on
                #    DVE first (a strided DMA lowers to 2-byte packets) --
                if t == T - 1:
                    hl = hn.rearrange("p (k c) -> p k c", k=HT)[
                        :, :, C - 1 : S : C]
                    hlc = sb.tile([128, HT * NB], BF16, name="hlc", tag="hlc")
                    nc.vector.tensor_copy(
                        hlc.rearrange("p (k n) -> p k n", k=HT), hl)
                    nc.sync.dma_start(out=hl_d[:, :], in_=hlc)

                # ---- s2 (ACT tail): sqrt (costs 2 act-table loads) ----
                for dt_, b, st in stages:
                    if dt_ == 2:
                        sq = sb.tile([4, 512], F32, name=f"sq{b}", tag="sq",
                                     bufs=2)
                        nc.scalar.activation(
                            out=sq, in_=st["dvar"], func=AF.Sqrt,
                            bias=eps_sb[0:4, :],
                            scale=1.0 / (float(H) * float(H)))
                        st["sq"] = sq

                # ---- yn stages (DVE tail): normalize one k-tile ----
                for dt_, b, st in stages:
                    if 5 <= dt_ <= 8:
                        k = dt_ - 5
                        yn = sb.tile([128, FB], BF16, name=f"yn{b}_{k}",
                                     tag="yn", bufs=2)
                        nc.vector.tensor_mul(
                            yn, st["y_all"][:, k * FB : (k + 1) * FB],
                            st["rs_bc"])
                        nc.vector.tensor_sub(yn, yn, st["mu_bc"])
                        if not triv_gb:
                            nc.gpsimd.tensor_scalar(
                                out=yn, in0=yn,
                                scalar1=gam_sb[:, k : k + 1],
                                scalar2=bet_sb[:, k : k + 1],
                                op0=ALU.mult, op1=ALU.add)
                        nc.sync.dma_start(
                            out=out_d[k, :, b * FB : (b + 1) * FB], in_=yn)
                        if k == HT - 1:
                            del ln[b]

                # ---- bc waves (PE tail): broadcast Smu/rs rows to
                #      [128,512] PSUM tiles via selector matmuls, borrowing
                #      the gh and z bank pairs (complete 1-mm groups) ----
                for dt_, b, st in stages:
                    if dt_ in (3, 4):
                        waves = (0, 1) if dt_ == 3 else (2, 3)
                        for i, g in enumerate(waves):
                            tg = ("gh0", "gh1") if i == 0 else ("r0", "r1")
                            mu_ps = rzp.tile([128, 512], F32,
                                             name=f"bcm{b}_{g}", tag=tg[0])
                            rs_ps = rzp.tile([128, 512], F32,
                                             name=f"bcr{b}_{g}", tag=tg[1])
                            nc.tensor.matmul(
                                mu_ps, sel_sb[0:4, g * 128 : (g + 1) * 128],
                                st["pkm"], start=True, stop=True,
                                skip_group_check=True)
                            nc.tensor.matmul(
                                rs_ps, sel_sb[0:4, g * 128 : (g + 1) * 128],
                                st["pkr"], start=True, stop=True,
                                skip_group_check=True)
                            nc.scalar.activation(
                                out=st["mu_bc"][:, g * 512 : (g + 1) * 512],
                                in_=mu_ps, func=AF.Identity, scale=1.0)
                            nc.scalar.activation(
                                out=st["rs_bc"][:, g * 512 : (g + 1) * 512],
                                in_=rs_ps, func=AF.Identity, scale=1.0)

                # ---- blk_end (PE tail): LN column-sum matmuls ----
                if blk_end:
                    stp0 = rzp.tile([128, 512], F32, name=f"st{blk}a",
                                    tag="gh0")
                    stp1 = rzp.tile([128, 512], F32, name=f"st{blk}b",
                                    tag="gh1")
                    for g in range(4):   # Smu rows 0..3 <- y cols g*512..
                        for k in range(HT):
                            nc.tensor.matmul(
                                stp0[0:4, :],
                                ind_sb[:, g * 4 : (g + 1) * 4],
                                y_all[:, k * FB + g * 512 : k * FB + g * 512 + 512],
                                start=(g == 0 and k == 0),
                                stop=(g == 3 and k == HT - 1),
                                skip_group_check=True)
                    for g in range(4):   # Sss rows 0..3 <- y2
                        for k in range(HT):
                            nc.tensor.matmul(
                                stp1[0:4, :],
                                ind_sb[:, g * 4 : (g + 1) * 4],
                                y2_all[:, k * FB + g * 512 : k * FB + g * 512 + 512],
                                start=(g == 0 and k == 0),
                                stop=(g == 3 and k == HT - 1),
                                skip_group_check=True)
                    ln[blk] = {"te": t, "stp0": stp0, "stp1": stp1,
                               "y_all": y_all}

                if t + 1 < T:
                    s_cur = s_nxt
                    xt = xt_nxt
                    gx_ps = gx_nxt

            # ---- tail: finish LN for the last block(s) ----
            for b in sorted(ln):
                st = ln[b]
                st_mu = sb.tile([4, 512], F32, name=f"tstm{b}", tag="stm",
                                bufs=2)
                nc.scalar.activation(
                    out=st_mu, in_=st["stp0"][0:4, :], func=AF.Identity,
                    scale=1.0)
                st_ss = sb.tile([4, 512], F32, name=f"tstv{b}", tag="stv",
                                bufs=2)
                nc.scalar.activation(
                    out=st_ss, in_=st["stp1"][0:4, :], func=AF.Identity,
                    scale=1.0)
                musq = sb.tile([4, 512], F32, name=f"tmq{b}", tag="mq", bufs=2)
                nc.vector.tensor_mul(musq, st_mu, st_mu)
                dvar = sb.tile([4, 512], F32, name=f"tdv{b}", tag="dv", bufs=2)
                nc.vector.scalar_tensor_tensor(
                    out=dvar, in0=st_ss, scalar=float(H), in1=musq,
                    op0=ALU.mult, op1=ALU.subtract)
                sq = sb.tile([4, 512], F32, name=f"tsq{b}", tag="sq", bufs=2)
                nc.scalar.activation(
                    out=sq, in_=dvar, func=AF.Sqrt,
                    bias=eps_sb[0:4, :],
                    scale=1.0 / (float(H) * float(H)))
                rst = sb.tile([4, 512], F32, name=f"trs{b}", tag="rs", bufs=2)
                nc.vector.reciprocal_approx_fast(out=rst, in_=sq)
                bsm = sb.tile([4, 512], F32, name=f"tbs{b}", tag="bs", bufs=2)
                nc.vector.scalar_tensor_tensor(
                    out=bsm, in0=st_mu, scalar=1.0 / float(H),
                    in1=rst, op0=ALU.mult, op1=ALU.mult)
                pkm = sb.tile([4, 512], BF16, name=f"tpkm{b}", tag="pkm",
                              bufs=2)
                nc.vector.tensor_copy(pkm, bsm)
                pkr = sb.tile([4, 512], BF16, name=f"tpkr{b}", tag="pkr",
                              bufs=2)
                nc.vector.tensor_copy(pkr, rst)
                mu_bc = sb.tile([128, FB], BF16, name=f"tmubc{b}", tag="mubc",
                                bufs=1)
                rs_bc = sb.tile([128, FB], BF16, name=f"trsbc{b}", tag="rsbc",
                                bufs=1)
                tags = [("gh0", "gh1"), ("z0", "z1"), ("r0", "r1"),
                        ("gx0", "gx1")]
                for g in range(4):
                    tg = tags[g]
                    pool = gxp if tg[0].startswith("gx") else rzp
                    mu_ps = rzp.tile([128, 512], F32, name=f"tbm{b}_{g}",
                                     tag=tg[0]) if tg[0] != "gx0" else                         gxp.tile([128, 512], F32, name=f"tbm{b}_{g}",
                                 tag=tg[0])
                    rs_ps = rzp.tile([128, 512], F32, name=f"tbr{b}_{g}",
                                     tag=tg[1]) if tg[1] != "gx1" else                         gxp.tile([128, 512], F32, name=f"tbr{b}_{g}",
                                 tag=tg[1])
                    nc.tensor.matmul(
                        mu_ps, sel_sb[0:4, g * 128 : (g + 1) * 128], pkm,
                        start=True, stop=True, skip_group_check=True)
                    nc.tensor.matmul(
                        rs_ps, sel_sb[0:4, g * 128 : (g + 1) * 128], pkr,
                        start=True, stop=True, skip_group_check=True)
                    nc.scalar.activation(
                        out=mu_bc[:, g * 512 : (g + 1) * 512], in_=mu_ps,
                        func=AF.Identity, scale=1.0)
                    nc.scalar.activation(
                        out=rs_bc[:, g * 512 : (g + 1) * 512], in_=rs_ps,
                        func=AF.Identity, scale=1.0)
                for k in range(HT):
                    yn = sb.tile([128, FB], BF16, name=f"tyn{b}_{k}",
                                 tag="yn", bufs=2)
                    nc.vector.tensor_mul(
                        yn, st["y_all"][:, k * FB : (k + 1) * FB], rs_bc)
                    nc.vector.tensor_sub(yn, yn, mu_bc)
                    if not triv_gb:
                        nc.gpsimd.tensor_scalar(
                            out=yn, in0=yn,
                            scalar1=gam_sb[:, k : k + 1],
                            scalar2=bet_sb[:, k : k + 1],
                            op0=ALU.mult, op1=ALU.add)
                    nc.sync.dma_start(
                        out=out_d[k, :, b * FB : (b + 1) * FB], in_=yn)
    nc.compile()
    return nc


def stage_inputs(input, h, is_initial, W_ih, W_hh, b_ih, b_hh, gamma, beta, R):
    """Host-side sharding/staging. Returns per-core input maps."""
    import ml_dtypes

    T = R + KS
    x = np.asarray(input, np.float32)
    h0 = np.asarray(h, np.float32)
    ii = np.asarray(is_initial).reshape(N, L)
    W_ih = np.asarray(W_ih, np.float32)
    W_hh = np.asarray(W_hh, np.float32)
    b_ih = np.asarray(b_ih, np.float32)
    b_hh = np.asarray(b_hh, np.float32)
    gamma = np.asarray(gamma, np.float32)
    beta = np.asarray(beta, np.float32)

    def bf(a):
        return np.ascontiguousarray(np.asarray(a, np.float32)).astype(
            ml_dtypes.bfloat16)

    mask = 1.0 - ii.astype(np.float32)  # [N, L]

    # l index per (c, t): warm-up reads the R steps before the chunk;
    # chunk 0's warm-up reads l in [KS-R, KS) (discarded garbage).
    l_for = np.empty((C, T), np.int64)
    for c in range(C):
        for t in range(T):
            l = c * KS + (t - R)
            l_for[c, t] = l if l >= 0 else l + KS

    wihT = np.ascontiguousarray(W_ih.T.reshape(HT, 128, 3 * H))
    whhT = np.ascontiguousarray(W_hh.T.reshape(HT, 128, 3 * H))
    brz = (b_ih + b_hh)[: 2 * H].reshape(8, 128).T.copy()        # [128, 8]
    bhn = b_hh[2 * H :].reshape(HT, 128).T.copy()                # [128, 4]
    binn = b_ih[2 * H :].reshape(HT, 128).T.copy()
    gam = gamma.reshape(HT, 128).T.copy()
    bet = beta.reshape(HT, 128).T.copy()
    # indicator stationary: group g (of 4) is a [128, 4] tile whose
    # column g is all-ones (routes a column-sum into PSUM partition g)
    ind = np.zeros((128, 16), np.float32)
    for g in range(4):
        ind[:, g * 4 + g] = 1.0
    # selector for the PSUM->all-partitions broadcast matmuls:
    # sel[c, g*128+po] = 1 iff c == g
    sel = np.zeros((4, 512), np.float32)
    for g in range(4):
        sel[g, g * 128 : (g + 1) * 128] = 1.0

    in_maps = []
    for core in range(NCORES):
        n0 = core * NB
        xc = x[n0 : n0 + NB]              # [NB, L, H]
        xg = xc[:, l_for, :]              # [NB, C, T, H]
        # xs2[t, p, k*S + s] with s = n*C + c, h = k*128 + p
        xs2 = np.ascontiguousarray(
            xg.transpose(2, 3, 0, 1)      # [T, H, NB, C]
            .reshape(T, HT, 128, S)
            .transpose(0, 2, 1, 3)        # [T, 128, HT, S]
            .reshape(T, 128, SF))
        mg = mask[n0 : n0 + NB][:, l_for]  # [NB, C, T]
        ms = np.ascontiguousarray(mg.transpose(2, 0, 1).reshape(T, S))
        m0 = mask[n0 : n0 + NB, 0]         # [NB]
        h0c = h0[n0 : n0 + NB] * m0[:, None]     # [NB, H]
        # h0m[p, k*NB + n] = h0c[n, k*128+p]
        h0m = np.ascontiguousarray(
            h0c.reshape(NB, HT, 128).transpose(2, 1, 0).reshape(128, HT * NB))
        in_maps.append({
            "xs": bf(xs2), "ms": bf(ms), "h0m": bf(h0m),
            "wih": bf(wihT), "whh": bf(whhT),
            "brz": brz, "bhn": bhn, "bin": binn,
            "gam": gam, "bet": bet, "ind": bf(ind), "sel": bf(sel),
        })
    return in_maps


def required_warmup(is_initial):
    """Max distance from a chunk boundary back to the latest reset."""
    ii = np.asarray(is_initial).reshape(N, L)
    need = 0
    for c in range(1, C):
        start = c * KS
        sub = ii[:, :start]
        for n in range(N):
            nz = np.nonzero(sub[n])[0]
            gap = start - nz[-1] if len(nz) else start
            need = max(need, gap)
    return need


def unstage_outputs(results):
    out = np.empty((N, L, H), np.float32)
    h_last = np.empty((N, H), np.float32)
    for core in range(NCORES):
        n0 = core * NB
        st = np.asarray(results[core]["out_st"], np.float32)  # [HT,128,KS*S]
        o = st.reshape(HT, 128, KS, NB, C).transpose(3, 4, 2, 0, 1)
        out[n0 : n0 + NB] = o.reshape(NB, L, H)
        hl = np.asarray(results[core]["hlast"], np.float32)   # [128, HT*NB]
        h_last[n0 : n0 + NB] = (
            hl.reshape(128, HT, NB).transpose(2, 1, 0).reshape(NB, H))
    h_exp = np.broadcast_to(h_last[:, None, :], (N, L, H)).copy()
    return out, h_exp


_PROGRAM_CACHE = {}


def kernel(input, h, is_initial, W_ih, W_hh, b_ih, b_hh, gamma, beta):
    need = required_warmup(is_initial)
    R = max(12, int(need))
    triv = bool(
        np.all(np.asarray(gamma) == 1.0) and np.all(np.asarray(beta) == 0.0))
    key = (R, triv)
    if key not in _PROGRAM_CACHE:
        _PROGRAM_CACHE[key] = build_program(R, triv_gb=triv)
    nc = _PROGRAM_CACHE[key]
    in_maps = stage_inputs(
        input, h, is_initial, W_ih, W_hh, b_ih, b_hh, gamma, beta, R)
    res = run_bass_kernel_spmd(nc, in_maps, list(range(NCORES))).results
    return unstage_outputs(res)


# revision 38
# speedup vs baseline: 1.1910x; 1.0048x over previous
"""Trainium2 Bass kernel for masked-GRU + residual + LayerNorm.

Problem: N=128 sequences of length L=512, hidden H=512.
  gx = x @ W_ih.T + b_ih            (precomputable input projection)
  per step l: hc = h * (1-is_initial[l]); gh = hc @ W_hh.T + b_hh
    r = sig(gx_r+gh_r); z = sig(gx_z+gh_z); n = tanh(gx_n + r*gh_n)
    h = (1-z)*n + z*hc
  out = LayerNorm(seq + x) * gamma + beta;  h_exp = broadcast(h_last)

Strategy (v2):
  * Data parallel: 16 batch rows per core (8 cores).
  * Sequence-chunk parallel: each L=512 sequence is split into C=16
    chunks of 32 steps, processed as independent columns, made exact by
    an R-step warm-up (a reset lands inside every R-window; verified at
    runtime). Chunk 0 injects the true h0 at entry to main.
  * bf16 everywhere off-PSUM: weights, x, gates, state, y, outputs.
    PSUM stays f32.  DVE runs 2-byte all-SBUF ops at 4x rate.
  * Packed state s [128, 1024] bf16 (4 h-tiles side by side in the free
    dim) so the elementwise chain runs as [128,512] halves.
  * PSUM banks (8): R pair (r gates), Z pair (z), G pair (gh_n),
    X pair (gx_n, lives across one step boundary).  LN stats borrow
    bank G0 right after it drains.
  * n-gate: gx_n stays in PSUM; w=(gh+bhh_n)*r on DVE, u=w+gx on Pool,
    tanh(u)+b_ih_n on ACT.  No identity-drain of gx.
  * LayerNorm per block of 8 steps, software-pipelined over the 7
    following steps so nothing blocks the recurrence: column sums via
    indicator-stationary matmuls into ONE [8,512] PSUM tile (rows 0-3
    Smu, rows 4-7 Sss), stats math as [4,512] multi-partition ops
    (D = H*Sss - Smu^2; rs = 1/sqrt(D + eps*H^2); out = ((H*y - Smu)
    * rs) * gamma + beta), DRAM-bounce broadcast of (Smu, rs) in bf16.
  * out / h_last stored bf16, upcast host-side.
"""
import sys

sys.path.insert(0, "/opt/trn_rl_repo")

import numpy as np

import concourse.bass as bass
import concourse.tile as tile
from concourse import bacc, mybir
from concourse.bass_utils import run_bass_kernel_spmd

F32 = mybir.dt.float32
BF16 = mybir.dt.bfloat16
AF = mybir.ActivationFunctionType
ALU = mybir.AluOpType

N, L, H = 128, 512, 512
NCORES = 8
NB = N // NCORES          # batch rows per core = 16
C = 16                    # chunks per sequence
KS = L // C               # main steps per chunk = 32
S = NB * C                # columns per core = 256
HT = H // 128             # h partition tiles = 4
BLK = 8                   # LN block (main steps)
NBLK = KS // BLK          # 4
FB = BLK * S              # 2048 block columns
SF = HT * S               # 1024 free dim of the packed state


def _bcast_ap(row_ap, parts=128, rep=1):
    """DRAM row AP -> partition-broadcast AP (0-stride over partitions,
    optionally replicated rep times along free)."""
    ap = [[0, parts]]
    if rep > 1:
        ap.append([0, rep])
    ap += [list(d) for d in row_ap.ap]
    return bass.AP(tensor=row_ap.tensor, offset=row_ap.offset, ap=ap)


def build_program(R=12, debug=False, triv_gb=True):
    T = R + KS
    nc = bacc.Bacc("TRN2", target_bir_lowering=False)

    xs_d = nc.declare_dram_parameter("xs", [T, 128, SF], BF16, isOutput=False)
    ms_d = nc.declare_dram_parameter("ms", [T, S], BF16, isOutput=False)
    h0m_d = nc.declare_dram_parameter("h0m", [128, HT * NB], BF16, isOutput=False)
    wih_d = nc.declare_dram_parameter("wih", [HT, 128, 3 * H], BF16, isOutput=False)
    whh_d = nc.declare_dram_parameter("whh", [HT, 128, 3 * H], BF16, isOutput=False)
    brz_d = nc.declare_dram_parameter("brz", [128, 8], F32, isOutput=False)
    bhn_d = nc.declare_dram_parameter("bhn", [128, HT], F32, isOutput=False)
    bin_d = nc.declare_dram_parameter("bin", [128, HT], F32, isOutput=False)
    gam_d = nc.declare_dram_parameter("gam", [128, HT], F32, isOutput=False)
    bet_d = nc.declare_dram_parameter("bet", [128, HT], F32, isOutput=False)
    ind_d = nc.declare_dram_parameter("ind", [128, 16], BF16, isOutput=False)
    sel_d = nc.declare_dram_parameter("sel", [4, 512], BF16, isOutput=False)

    out_d = nc.declare_dram_parameter("out_st", [HT, 128, KS * S], BF16, isOutput=True)
    if debug:
        dbg_names = ["mk0", "r0", "z0", "n0", "hn0", "s1", "s2x", "hnT"]
        dbg_d = {
            nm: nc.declare_dram_parameter(f"dbg_{nm}", [128, SF], BF16,
                                          isOutput=True)
            for nm in dbg_names
        }
    hl_d = nc.declare_dram_parameter("hlast", [128, HT * NB], BF16, isOutput=True)


    with tile.TileContext(nc) as tc:
        with (
            tc.tile_pool(name="const", bufs=1) as cst,
            tc.tile_pool(name="sb", bufs=1) as sb,
            tc.tile_pool(name="rzp", bufs=1, space="PSUM") as rzp,
            tc.tile_pool(name="gxp", bufs=1, space="PSUM") as gxp,
        ):
            # ---- constants (wih first: needed by the t=0 prefill) ----
            x0 = sb.tile([128, SF], BF16, name="xt0", tag="xt", bufs=4)
            nc.sync.dma_start(out=x0, in_=xs_d[0, :, :])
            wih_sb, whh_sb = [], []
            for k in range(HT):
                w1 = cst.tile([128, 3 * H], BF16, name=f"wih_sb{k}", tag=f"wih{k}")
                (nc.sync if k % 2 else nc.scalar).dma_start(
                    out=w1, in_=wih_d[k, :, :])
                wih_sb.append(w1)
            for k in range(HT):
                w2 = cst.tile([128, 3 * H], BF16, name=f"whh_sb{k}", tag=f"whh{k}")
                (nc.sync if k % 2 else nc.scalar).dma_start(
                    out=w2, in_=whh_d[k, :, :])
                whh_sb.append(w2)
            h0m_sb = cst.tile([128, HT * NB], BF16, name="h0m_sb", tag="h0m")
            nc.sync.dma_start(out=h0m_sb, in_=h0m_d[:, :])
            brz_sb = cst.tile([128, 8], F32, name="brz_sb", tag="brz")
            nc.sync.dma_start(out=brz_sb, in_=brz_d[:, :])
            bhn_sb = cst.tile([128, HT], F32, name="bhn_sb", tag="bhn")
            nc.sync.dma_start(out=bhn_sb, in_=bhn_d[:, :])
            bin_sb = cst.tile([128, HT], F32, name="bin_sb", tag="bin")
            nc.sync.dma_start(out=bin_sb, in_=bin_d[:, :])
            gam_sb = cst.tile([128, HT], F32, name="gam_sb", tag="gam")
            nc.sync.dma_start(out=gam_sb, in_=gam_d[:, :])
            bet_sb = cst.tile([128, HT], F32, name="bet_sb", tag="bet")
            nc.sync.dma_start(out=bet_sb, in_=bet_d[:, :])
            ind_sb = cst.tile([128, 16], BF16, name="ind_sb", tag="ind")
            nc.sync.dma_start(out=ind_sb, in_=ind_d[:, :])
            eps_sb = cst.tile([128, 1], F32, name="eps_sb", tag="eps")
            nc.vector.memset(eps_sb, 1e-5)
            sel_sb = cst.tile([4, 512], BF16, name="sel_sb", tag="sel")
            nc.sync.dma_start(out=sel_sb, in_=sel_d[:, :])
            # masks: load once to partition 0, broadcast on Pool
            ms_all = cst.tile([128, T * S], BF16, name="ms_all", tag="msb")
            nc.sync.dma_start(
                out=ms_all[0:1, :], in_=ms_d[:, :].rearrange("t s -> (t s)"))
            nc.gpsimd.partition_broadcast(ms_all, ms_all[0:1, :])

            # ---- initial (zero) state ----
            s_cur = sb.tile([128, SF], BF16, name="s_init", tag="state", bufs=3)
            nc.vector.memset(s_cur, 0.0)

            def load_x(t):
                xt = sb.tile([128, SF], BF16, name=f"xt{t}", tag="xt", bufs=4)
                nc.sync.dma_start(out=xt, in_=xs_d[t, :, :])
                return xt



            def prefill_gx(t, xt):
                # open+close gx_n groups for step t (wih only); X pair
                # [gx8|gx9], [gx10|gx11]; stays in PSUM until step t's u.
                gx_ps = [
                    gxp.tile([128, 512], F32, name=f"gx{t}_{j}", tag=f"gx{j}")
                    for j in range(2)
                ]
                for k4 in range(4):
                    j = 8 + k4
                    oap = gx_ps[k4 // 2][:, (k4 % 2) * 256 : (k4 % 2) * 256 + 256]
                    for k in range(HT):
                        nc.tensor.matmul(
                            oap, wih_sb[k][:, j * 128 : (j + 1) * 128],
                            xt[:, k * 256 : (k + 1) * 256],
                            start=(k == 0), stop=(k == HT - 1))
                return gx_ps

            xt = x0
            gx_ps = prefill_gx(0, xt)

            # LN pipeline state, keyed by block id
            ln = {}
            y_all = y2_all = None

            for t in range(T):
                main = t >= R
                toff = (t - R) % BLK
                blk = (t - R) // BLK
                blk_end = main and toff == BLK - 1

                # LN pipeline stage for earlier blocks this iteration
                stages = [(t - st["te"], b, st) for b, st in list(ln.items())
                          if 1 <= t - st["te"] <= 8]

                # ---- s1 (ACT top): drain the stats PSUM banks ----
                for dt_, b, st in stages:
                    if dt_ == 1:
                        st_mu = sb.tile([4, 512], F32, name=f"stm{b}",
                                        tag="stm", bufs=2)
                        nc.scalar.activation(
                            out=st_mu, in_=st["stp0"][0:4, :],
                            func=AF.Identity, scale=1.0)
                        st_ss = sb.tile([4, 512], F32, name=f"stv{b}",
                                        tag="stv", bufs=2)
                        nc.vector.tensor_copy(st_ss, st["stp1"][0:4, :])
                        st["st_mu"] = st_mu
                        st["st_ss"] = st_ss

                if t + 1 < T:
                    xt_nxt = load_x(t + 1)
                    mk = ms_all[:, (t + 1) * S : (t + 2) * S]

                # ---- s1 (DVE top): musq, dvar ----
                for dt_, b, st in stages:
                    if dt_ == 1:
                        musq = sb.tile([4, 512], F32, name=f"mq{b}", tag="mq",
                                       bufs=2)
                        nc.vector.tensor_mul(
                            musq, st["st_mu"], st["st_mu"])
                        dvar = sb.tile([4, 512], F32, name=f"dv{b}", tag="dv",
                                       bufs=2)
                        nc.vector.scalar_tensor_tensor(
                            out=dvar, in0=st["st_ss"],
                            scalar=float(H), in1=musq,
                            op0=ALU.mult, op1=ALU.subtract)
                        st["dvar"] = dvar

                # ---- s3 (DVE top): recip + bf16 packs ----
                for dt_, b, st in stages:
                    if dt_ == 3:
                        rst = sb.tile([4, 512], F32, name=f"rs{b}", tag="rs",
                                      bufs=2)
                        nc.vector.reciprocal_approx_fast(out=rst, in_=st["sq"])
                        bsm = sb.tile([4, 512], F32, name=f"bs{b}", tag="bs",
                                      bufs=2)
                        nc.vector.scalar_tensor_tensor(
                            out=bsm, in0=st["st_mu"], scalar=1.0 / float(H),
                            in1=rst, op0=ALU.mult, op1=ALU.mult)
                        pkm = sb.tile([4, 512], BF16, name=f"pkm{b}",
                                      tag="pkm", bufs=2)
                        nc.vector.tensor_copy(pkm, bsm)
                        pkr = sb.tile([4, 512], BF16, name=f"pkr{b}",
                                      tag="pkr", bufs=2)
                        nc.vector.tensor_copy(pkr, rst)
                        st["pkm"] = pkm
                        st["pkr"] = pkr
                        st["mu_bc"] = sb.tile([128, FB], BF16, name=f"mubc{b}",
                                              tag="mubc", bufs=1)
                        st["rs_bc"] = sb.tile([128, FB], BF16, name=f"rsbc{b}",
                                              tag="rsbc", bufs=1)



                # ---- PE: all r/z/gh groups open and close within this
                #      iteration (cross-iteration open groups on sliced
                #      tiles miscompile).  Order: independent wih opens
                #      first, state-dependent whh closes mid-stream, the
                #      complete gx prefill for t+1 last. ----
                r_ps = [
                    rzp.tile([128, 512], F32, name=f"r{t}_{j}", tag=f"r{j}")
                    for j in range(2)
                ]
                gh_ps = [
                    rzp.tile([128, 512], F32, name=f"gh{t}_{j}", tag=f"gh{j}")
                    for j in range(2)
                ]
                z_ps = [
                    rzp.tile([128, 512], F32, name=f"z{t}_{j}", tag=f"z{j}")
                    for j in range(2)
                ]

                def wih_open(ps, j4, j0):
                    # opens the j4 slice group (start zeroes the bank's
                    # write-bitmap: no other start may hit this bank until
                    # this group fully closes)
                    j = j0 + j4
                    oap = ps[j4 // 2][:, (j4 % 2) * 256 : (j4 % 2) * 256 + 256]
                    for k in range(HT):
                        nc.tensor.matmul(
                            oap, wih_sb[k][:, j * 128 : (j + 1) * 128],
                            xt[:, k * 256 : (k + 1) * 256],
                            start=(k == 0), stop=False)

                def whh_close(ps, j4, j0):
                    j = j0 + j4
                    oap = ps[j4 // 2][:, (j4 % 2) * 256 : (j4 % 2) * 256 + 256]
                    for k in range(HT):
                        nc.tensor.matmul(
                            oap, whh_sb[k][:, j * 128 : (j + 1) * 128],
                            s_cur[:, k * 256 : (k + 1) * 256],
                            start=False, stop=(k == HT - 1))

                def gh_bank(h):
                    for k4 in (2 * h, 2 * h + 1):
                        j = 8 + k4
                        oap = gh_ps[h][:, (k4 % 2) * 256 : (k4 % 2) * 256 + 256]
                        for k in range(HT):
                            nc.tensor.matmul(
                                oap, whh_sb[k][:, j * 128 : (j + 1) * 128],
                                s_cur[:, k * 256 : (k + 1) * 256],
                                start=(k == 0), stop=(k == HT - 1))

                # independent x-projections first (one open per bank),
                # then per-bank sequential close/open/close
                wih_open(r_ps, 0, 0)
                wih_open(r_ps, 2, 0)
                wih_open(z_ps, 0, 4)
                wih_open(z_ps, 2, 4)
                whh_close(r_ps, 0, 0)
                wih_open(r_ps, 1, 0)
                whh_close(r_ps, 1, 0)
                whh_close(r_ps, 2, 0)
                wih_open(r_ps, 3, 0)
                whh_close(r_ps, 3, 0)
                gh_bank(0)
                gh_bank(1)
                whh_close(z_ps, 0, 4)
                wih_open(z_ps, 1, 4)
                whh_close(z_ps, 1, 4)
                whh_close(z_ps, 2, 4)
                wih_open(z_ps, 3, 4)
                whh_close(z_ps, 3, 4)
                if t + 1 < T:
                    gx_nxt = prefill_gx(t + 1, xt_nxt)

                # ---- ACT: r sigmoids (bf16 out) ----
                r_t = sb.tile([128, SF], BF16, name=f"rt{t}", tag="rt", bufs=2)
                for k in range(HT):
                    nc.scalar.activation(
                        out=r_t[:, k * 256 : (k + 1) * 256],
                        in_=r_ps[k // 2][:, (k % 2) * 256 : (k % 2) * 256 + 256],
                        func=AF.Sigmoid, bias=brz_sb[:, k : k + 1], scale=1.0)

                # ---- DVE: w = (gh + bhn) * r  (per k-tile) ----
                w_t = sb.tile([128, SF], BF16, name=f"wt{t}", tag="wt", bufs=2)
                for k in range(HT):
                    nc.vector.scalar_tensor_tensor(
                        out=w_t[:, k * 256 : (k + 1) * 256],
                        in0=gh_ps[k // 2][:, (k % 2) * 256 : (k % 2) * 256 + 256],
                        scalar=bhn_sb[:, k : k + 1],
                        in1=r_t[:, k * 256 : (k + 1) * 256],
                        op0=ALU.add, op1=ALU.mult)

                # ---- DVE: u = w + gx (per X bank, [128,512]);
                #      Pool cannot read PSUM on TRN2 ----
                u_t = sb.tile([128, SF], BF16, name=f"ut{t}", tag="ut", bufs=2)
                for h in range(2):
                    nc.vector.tensor_add(
                        u_t[:, h * 512 : (h + 1) * 512],
                        w_t[:, h * 512 : (h + 1) * 512], gx_ps[h])

                # ---- ACT: tanh (per k-tile, bias=b_ih_n) / z sigmoids ----
                n_t = sb.tile([128, SF], BF16, name=f"nt{t}", tag="nt", bufs=2)
                z_t = sb.tile([128, SF], BF16, name=f"zt{t}", tag="zt", bufs=2)

                def tanh_k(k):
                    nc.scalar.activation(
                        out=n_t[:, k * 256 : (k + 1) * 256],
                        in_=u_t[:, k * 256 : (k + 1) * 256],
                        func=AF.Tanh, bias=bin_sb[:, k : k + 1], scale=1.0)

                def zsig_k(k):
                    nc.scalar.activation(
                        out=z_t[:, k * 256 : (k + 1) * 256],
                        in_=z_ps[k // 2][:, (k % 2) * 256 : (k % 2) * 256 + 256],
                        func=AF.Sigmoid, bias=brz_sb[:, 4 + k : 5 + k], scale=1.0)

                tanh_k(0)
                tanh_k(1)
                zsig_k(0)
                zsig_k(1)
                tanh_k(2)
                tanh_k(3)
                zsig_k(2)
                zsig_k(3)

                # ---- DVE chain (two [128,512] halves):
                #      t1 = s - n; t1 *= z; hn = t1 + n; s' = hn*mk;
                #      y = hn + x; y2 = y*y ----
                hn = sb.tile([128, SF], BF16, name=f"hn{t}", tag="hn", bufs=2)
                t1 = sb.tile([128, SF], BF16, name=f"t1{t}", tag="t1", bufs=2)
                s_nxt = None
                if t + 1 < T:
                    s_nxt = sb.tile([128, SF], BF16, name=f"s{t + 1}",
                                    tag="state", bufs=3)
                if main and toff == 0:
                    y_all = sb.tile([128, HT * FB], BF16,
                                    name=f"y{blk}", tag="y_all", bufs=2)
                    y2_all = sb.tile([128, HT * FB], BF16,
                                     name=f"y2{blk}", tag="y2_all", bufs=1)

                for h in range(2):
                    sl = slice(h * 512, (h + 1) * 512)
                    nc.vector.tensor_sub(t1[:, sl], s_cur[:, sl], n_t[:, sl])
                    nc.vector.tensor_mul(t1[:, sl], t1[:, sl], z_t[:, sl])
                    nc.vector.tensor_add(hn[:, sl], t1[:, sl], n_t[:, sl])
                    if s_nxt is not None:
                        for k in (2 * h, 2 * h + 1):
                            ksl = slice(k * 256, (k + 1) * 256)
                            nc.vector.tensor_mul(
                                s_nxt[:, ksl], hn[:, ksl], mk)
                if main:
                    for k in range(HT):
                        o = k * FB + toff * S
                        ksl = slice(k * 256, (k + 1) * 256)
                        nc.vector.tensor_add(
                            y_all[:, o : o + S], hn[:, ksl], xt[:, ksl])
                        nc.gpsimd.tensor_mul(
                            y2_all[:, o : o + S],
                            y_all[:, o : o + S], y_all[:, o : o + S])

                if debug and t == 0:
                    pass
                    nc.sync.dma_start(out=dbg_d["r0"][:, :], in_=r_t)
                    nc.sync.dma_start(out=dbg_d["z0"][:, :], in_=z_t)
                    nc.sync.dma_start(out=dbg_d["n0"][:, :], in_=n_t)
                    nc.sync.dma_start(out=dbg_d["hn0"][:, :], in_=hn)
                    nc.sync.dma_start(out=dbg_d["s1"][:, :], in_=s_nxt)
                if debug and t == 1:
                    nc.sync.dma_start(out=dbg_d["s2x"][:, :], in_=s_nxt)
                if debug and t == T - 1:
                    nc.sync.dma_start(out=dbg_d["hnT"][:, :], in_=hn)

                # -- h0 injection at entry to main (chunk-0 columns) --
                if t + 1 == R:
                    inj = s_nxt.rearrange("p (k c) -> p k c", k=HT)[:, :, 0:S:C]
                    nc.vector.tensor_copy(
                        inj, h0m_sb.rearrange("p (k n) -> p k n", k=HT))

                # -- final hidden state (chunk C-1 columns); compact on
                #    DVE first (a strided DMA lowers to 2-byte packets) --
                if t == T - 1:
                    hl = hn.rearrange("p (k c) -> p k c", k=HT)[
                        :, :, C - 1 : S : C]
                    hlc = sb.tile([128, HT * NB], BF16, name="hlc", tag="hlc")
                    nc.vector.tensor_copy(
                        hlc.rearrange("p (k n) -> p k n", k=HT), hl)
                    nc.sync.dma_start(out=hl_d[:, :], in_=hlc)

                # ---- s2 (ACT tail): sqrt (costs 2 act-table loads) ----
                for dt_, b, st in stages:
                    if dt_ == 2:
                        sq = sb.tile([4, 512], F32, name=f"sq{b}", tag="sq",
                                     bufs=2)
                        nc.scalar.activation(
                            out=sq, in_=st["dvar"], func=AF.Sqrt,
                            bias=eps_sb[0:4, :],
                            scale=1.0 / (float(H) * float(H)))
                        st["sq"] = sq

                # ---- yn stages (DVE tail): normalize one k-tile ----
                for dt_, b, st in stages:
                    if 5 <= dt_ <= 8:
                        k = dt_ - 5
                        yn = sb.tile([128, FB], BF16, name=f"yn{b}_{k}",
                                     tag="yn", bufs=3)
                        nc.vector.tensor_mul(
                            yn, st["y_all"][:, k * FB : (k + 1) * FB],
                            st["rs_bc"])
                        nc.vector.tensor_sub(yn, yn, st["mu_bc"])
                        if not triv_gb:
                            nc.gpsimd.tensor_scalar(
                                out=yn, in0=yn,
                                scalar1=gam_sb[:, k : k + 1],
                                scalar2=bet_sb[:, k : k + 1],
                                op0=ALU.mult, op1=ALU.add)
                        (nc.sync if k % 2 else nc.scalar).dma_start(
                            out=out_d[k, :, b * FB : (b + 1) * FB], in_=yn)
                        if k == HT - 1:
                            del ln[b]

                # ---- bc waves (PE tail): broadcast Smu/rs rows to
                #      [128,512] PSUM tiles via selector matmuls, borrowing
                #      the gh and z bank pairs (complete 1-mm groups) ----
                for dt_, b, st in stages:
                    if dt_ in (3, 4):
                        waves = (0, 1) if dt_ == 3 else (2, 3)
                        for i, g in enumerate(waves):
                            tg = ("gh0", "gh1") if i == 0 else ("r0", "r1")
                            mu_ps = rzp.tile([128, 512], F32,
                                             name=f"bcm{b}_{g}", tag=tg[0])
                            rs_ps = rzp.tile([128, 512], F32,
                                             name=f"bcr{b}_{g}", tag=tg[1])
                            nc.tensor.matmul(
                                mu_ps, sel_sb[0:4, g * 128 : (g + 1) * 128],
                                st["pkm"], start=True, stop=True,
                                skip_group_check=True)
                            nc.tensor.matmul(
                                rs_ps, sel_sb[0:4, g * 128 : (g + 1) * 128],
                                st["pkr"], start=True, stop=True,
                                skip_group_check=True)
                            nc.scalar.activation(
                                out=st["mu_bc"][:, g * 512 : (g + 1) * 512],
                                in_=mu_ps, func=AF.Identity, scale=1.0)
                            nc.scalar.activation(
                                out=st["rs_bc"][:, g * 512 : (g + 1) * 512],
                                in_=rs_ps, func=AF.Identity, scale=1.0)

                # ---- blk_end (PE tail): LN column-sum matmuls ----
                if blk_end:
                    stp0 = rzp.tile([128, 512], F32, name=f"st{blk}a",
                                    tag="gh0")
                    stp1 = rzp.tile([128, 512], F32, name=f"st{blk}b",
                                    tag="gh1")
                    for g in range(4):   # Smu rows 0..3 <- y cols g*512..
                        for k in range(HT):
                            nc.tensor.matmul(
                                stp0[0:4, :],
                                ind_sb[:, g * 4 : (g + 1) * 4],
                                y_all[:, k * FB + g * 512 : k * FB + g * 512 + 512],
                                start=(g == 0 and k == 0),
                                stop=(g == 3 and k == HT - 1),
                                skip_group_check=True)
                    for g in range(4):   # Sss rows 0..3 <- y2
                        for k in range(HT):
                            nc.tensor.matmul(
                                stp1[0:4, :],
                                ind_sb[:, g * 4 : (g + 1) * 4],
                                y2_all[:, k * FB + g * 512 : k * FB + g * 512 + 512],
                                start=(g == 0 and k == 0),
                                stop=(g == 3 and k == HT - 1),
                                skip_group_check=True)
                    ln[blk] = {"te": t, "stp0": stp0, "stp1": stp1,
                               "y_all": y_all}

                if t + 1 < T:
                    s_cur = s_nxt
                    xt = xt_nxt
                    gx_ps = gx_nxt

            # ---- tail: finish LN for the last block(s) ----
            for b in sorted(ln):
                st = ln[b]
                st_mu = sb.tile([4, 512], F32, name=f"tstm{b}", tag="stm",
                                bufs=2)
                nc.scalar.activation(
                    out=st_mu, in_=st["stp0"][0:4, :], func=AF.Identity,
                    scale=1.0)
                st_ss = sb.tile([4, 512], F32, name=f"tstv{b}", tag="stv",
                                bufs=2)
                nc.scalar.activation(
                    out=st_ss, in_=st["stp1"][0:4, :], func=AF.Identity,
                    scale=1.0)
                musq = sb.tile([4, 512], F32, name=f"tmq{b}", tag="mq", bufs=2)
                nc.vector.tensor_mul(musq, st_mu, st_mu)
                dvar = sb.tile([4, 512], F32, name=f"tdv{b}", tag="dv", bufs=2)
                nc.vector.scalar_tensor_tensor(
                    out=dvar, in0=st_ss, scalar=float(H), in1=musq,
                    op0=ALU.mult, op1=ALU.subtract)
                sq = sb.tile([4, 512], F32, name=f"tsq{b}", tag="sq", bufs=2)
                nc.scalar.activation(
                    out=sq, in_=dvar, func=AF.Sqrt,
                    bias=eps_sb[0:4, :],
                    scale=1.0 / (float(H) * float(H)))
                rst = sb.tile([4, 512], F32, name=f"trs{b}", tag="rs", bufs=2)
                nc.vector.reciprocal_approx_fast(out=rst, in_=sq)
                bsm = sb.tile([4, 512], F32, name=f"tbs{b}", tag="bs", bufs=2)
                nc.vector.scalar_tensor_tensor(
                    out=bsm, in0=st_mu, scalar=1.0 / float(H),
                    in1=rst, op0=ALU.mult, op1=ALU.mult)
                pkm = sb.tile([4, 512], BF16, name=f"tpkm{b}", tag="pkm",
                              bufs=2)
                nc.vector.tensor_copy(pkm, bsm)
                pkr = sb.tile([4, 512], BF16, name=f"tpkr{b}", tag="pkr",
                              bufs=2)
                nc.vector.tensor_copy(pkr, rst)
                mu_bc = sb.tile([128, FB], BF16, name=f"tmubc{b}", tag="mubc",
                                bufs=1)
                rs_bc = sb.tile([128, FB], BF16, name=f"trsbc{b}", tag="rsbc",
                                bufs=1)
                tags = [("gh0", "gh1"), ("z0", "z1"), ("r0", "r1"),
                        ("gx0", "gx1")]
                for g in range(4):
                    tg = tags[g]
                    pool = gxp if tg[0].startswith("gx") else rzp
                    mu_ps = rzp.tile([128, 512], F32, name=f"tbm{b}_{g}",
                                     tag=tg[0]) if tg[0] != "gx0" else                         gxp.tile([128, 512], F32, name=f"tbm{b}_{g}",
                                 tag=tg[0])
                    rs_ps = rzp.tile([128, 512], F32, name=f"tbr{b}_{g}",
                                     tag=tg[1]) if tg[1] != "gx1" else                         gxp.tile([128, 512], F32, name=f"tbr{b}_{g}",
                                 tag=tg[1])
                    nc.tensor.matmul(
                        mu_ps, sel_sb[0:4, g * 128 : (g + 1) * 128], pkm,
                        start=True, stop=True, skip_group_check=True)
                    nc.tensor.matmul(
                        rs_ps, sel_sb[0:4, g * 128 : (g + 1) * 128], pkr,
                        start=True, stop=True, skip_group_check=True)
                    nc.scalar.activation(
                        out=mu_bc[:, g * 512 : (g + 1) * 512], in_=mu_ps,
                        func=AF.Identity, scale=1.0)
                    nc.scalar.activation(
                        out=rs_bc[:, g * 512 : (g + 1) * 512], in_=rs_ps,
                        func=AF.Identity, scale=1.0)
                for k in range(HT):
                    yn = sb.tile([128, FB], BF16, name=f"tyn{b}_{k}",
                                 tag="yn", bufs=3)
                    nc.vector.tensor_mul(
                        yn, st["y_all"][:, k * FB : (k + 1) * FB], rs_bc)
                    nc.vector.tensor_sub(yn, yn, mu_bc)
                    if not triv_gb:
                        nc.gpsimd.tensor_scalar(
                            out=yn, in0=yn,
                            scalar1=gam_sb[:, k : k + 1],
                            scalar2=bet_sb[:, k : k + 1],
                            op0=ALU.mult, op1=ALU.add)
                    (nc.sync if k % 2 else nc.scalar).dma_start(
                        out=out_d[k, :, b * FB : (b + 1) * FB], in_=yn)
    nc.compile()
    return nc


def stage_inputs(input, h, is_initial, W_ih, W_hh, b_ih, b_hh, gamma, beta, R):
    """Host-side sharding/staging. Returns per-core input maps."""
    import ml_dtypes

    T = R + KS
    x = np.asarray(input, np.float32)
    h0 = np.asarray(h, np.float32)
    ii = np.asarray(is_initial).reshape(N, L)
    W_ih = np.asarray(W_ih, np.float32)
    W_hh = np.asarray(W_hh, np.float32)
    b_ih = np.asarray(b_ih, np.float32)
    b_hh = np.asarray(b_hh, np.float32)
    gamma = np.asarray(gamma, np.float32)
    beta = np.asarray(beta, np.float32)

    def bf(a):
        return np.ascontiguousarray(np.asarray(a, np.float32)).astype(
            ml_dtypes.bfloat16)

    mask = 1.0 - ii.astype(np.float32)  # [N, L]

    # l index per (c, t): warm-up reads the R steps before the chunk;
    # chunk 0's warm-up reads l in [KS-R, KS) (discarded garbage).
    l_for = np.empty((C, T), np.int64)
    for c in range(C):
        for t in range(T):
            l = c * KS + (t - R)
            l_for[c, t] = l if l >= 0 else l + KS

    wihT = np.ascontiguousarray(W_ih.T.reshape(HT, 128, 3 * H))
    whhT = np.ascontiguousarray(W_hh.T.reshape(HT, 128, 3 * H))
    brz = (b_ih + b_hh)[: 2 * H].reshape(8, 128).T.copy()        # [128, 8]
    bhn = b_hh[2 * H :].reshape(HT, 128).T.copy()                # [128, 4]
    binn = b_ih[2 * H :].reshape(HT, 128).T.copy()
    gam = gamma.reshape(HT, 128).T.copy()
    bet = beta.reshape(HT, 128).T.copy()
    # indicator stationary: group g (of 4) is a [128, 4] tile whose
    # column g is all-ones (routes a column-sum into PSUM partition g)
    ind = np.zeros((128, 16), np.float32)
    for g in range(4):
        ind[:, g * 4 + g] = 1.0
    # selector for the PSUM->all-partitions broadcast matmuls:
    # sel[c, g*128+po] = 1 iff c == g
    sel = np.zeros((4, 512), np.float32)
    for g in range(4):
        sel[g, g * 128 : (g + 1) * 128] = 1.0

    in_maps = []
    for core in range(NCORES):
        n0 = core * NB
        xc = x[n0 : n0 + NB]              # [NB, L, H]
        xg = xc[:, l_for, :]              # [NB, C, T, H]
        # xs2[t, p, k*S + s] with s = n*C + c, h = k*128 + p
        xs2 = np.ascontiguousarray(
            xg.transpose(2, 3, 0, 1)      # [T, H, NB, C]
            .reshape(T, HT, 128, S)
            .transpose(0, 2, 1, 3)        # [T, 128, HT, S]
            .reshape(T, 128, SF))
        mg = mask[n0 : n0 + NB][:, l_for]  # [NB, C, T]
        ms = np.ascontiguousarray(mg.transpose(2, 0, 1).reshape(T, S))
        m0 = mask[n0 : n0 + NB, 0]         # [NB]
        h0c = h0[n0 : n0 + NB] * m0[:, None]     # [NB, H]
        # h0m[p, k*NB + n] = h0c[n, k*128+p]
        h0m = np.ascontiguousarray(
            h0c.reshape(NB, HT, 128).transpose(2, 1, 0).reshape(128, HT * NB))
        in_maps.append({
            "xs": bf(xs2), "ms": bf(ms), "h0m": bf(h0m),
            "wih": bf(wihT), "whh": bf(whhT),
            "brz": brz, "bhn": bhn, "bin": binn,
            "gam": gam, "bet": bet, "ind": bf(ind), "sel": bf(sel),
        })
    return in_maps


def required_warmup(is_initial):
    """Max distance from a chunk boundary back to the latest reset."""
    ii = np.asarray(is_initial).reshape(N, L)
    need = 0
    for c in range(1, C):
        start = c * KS
        sub = ii[:, :start]
        for n in range(N):
            nz = np.nonzero(sub[n])[0]
            gap = start - nz[-1] if len(nz) else start
            need = max(need, gap)
    return need


def unstage_outputs(results):
    out = np.empty((N, L, H), np.float32)
    h_last = np.empty((N, H), np.float32)
    for core in range(NCORES):
        n0 = core * NB
        st = np.asarray(results[core]["out_st"], np.float32)  # [HT,128,KS*S]
        o = st.reshape(HT, 128, KS, NB, C).transpose(3, 4, 2, 0, 1)
        out[n0 : n0 + NB] = o.reshape(NB, L, H)
        hl = np.asarray(results[core]["hlast"], np.float32)   # [128, HT*NB]
        h_last[n0 : n0 + NB] = (
            hl.reshape(128, HT, NB).transpose(2, 1, 0).reshape(NB, H))
    h_exp = np.broadcast_to(h_last[:, None, :], (N, L, H)).copy()
    return out, h_exp


_PROGRAM_CACHE = {}


def kernel(input, h, is_initial, W_ih, W_hh, b_ih, b_hh, gamma, beta):
    need = required_warmup(is_initial)
    R = max(12, int(need))
    triv = bool(
        np.all(np.asarray(gamma) == 1.0) and np.all(np.asarray(beta) == 0.0))
    key = (R, triv)
    if key not in _PROGRAM_CACHE:
        _PROGRAM_CACHE[key] = build_program(R, triv_gb=triv)
    nc = _PROGRAM_CACHE[key]
    in_maps = stage_inputs(
        input, h, is_initial, W_ih, W_hh, b_ih, b_hh, gamma, beta, R)
    res = run_bass_kernel_spmd(nc, in_maps, list(range(NCORES))).results
    return unstage_outputs(res)


# revision 39
# speedup vs baseline: 1.1940x; 1.0025x over previous
"""Trainium2 Bass kernel for masked-GRU + residual + LayerNorm.

Problem: N=128 sequences of length L=512, hidden H=512.
  gx = x @ W_ih.T + b_ih            (precomputable input projection)
  per step l: hc = h * (1-is_initial[l]); gh = hc @ W_hh.T + b_hh
    r = sig(gx_r+gh_r); z = sig(gx_z+gh_z); n = tanh(gx_n + r*gh_n)
    h = (1-z)*n + z*hc
  out = LayerNorm(seq + x) * gamma + beta;  h_exp = broadcast(h_last)

Strategy (v2):
  * Data parallel: 16 batch rows per core (8 cores).
  * Sequence-chunk parallel: each L=512 sequence is split into C=16
    chunks of 32 steps, processed as independent columns, made exact by
    an R-step warm-up (a reset lands inside every R-window; verified at
    runtime). Chunk 0 injects the true h0 at entry to main.
  * bf16 everywhere off-PSUM: weights, x, gates, state, y, outputs.
    PSUM stays f32.  DVE runs 2-byte all-SBUF ops at 4x rate.
  * Packed state s [128, 1024] bf16 (4 h-tiles side by side in the free
    dim) so the elementwise chain runs as [128,512] halves.
  * PSUM banks (8): R pair (r gates), Z pair (z), G pair (gh_n),
    X pair (gx_n, lives across one step boundary).  LN stats borrow
    bank G0 right after it drains.
  * n-gate: gx_n stays in PSUM; w=(gh+bhh_n)*r on DVE, u=w+gx on Pool,
    tanh(u)+b_ih_n on ACT.  No identity-drain of gx.
  * LayerNorm per block of 8 steps, software-pipelined over the 7
    following steps so nothing blocks the recurrence: column sums via
    indicator-stationary matmuls into ONE [8,512] PSUM tile (rows 0-3
    Smu, rows 4-7 Sss), stats math as [4,512] multi-partition ops
    (D = H*Sss - Smu^2; rs = 1/sqrt(D + eps*H^2); out = ((H*y - Smu)
    * rs) * gamma + beta), DRAM-bounce broadcast of (Smu, rs) in bf16.
  * out / h_last stored bf16, upcast host-side.
"""
import sys

sys.path.insert(0, "/opt/trn_rl_repo")

import numpy as np

import concourse.bass as bass
import concourse.tile as tile
from concourse import bacc, mybir
from concourse.bass_utils import run_bass_kernel_spmd

F32 = mybir.dt.float32
BF16 = mybir.dt.bfloat16
AF = mybir.ActivationFunctionType
ALU = mybir.AluOpType

N, L, H = 128, 512, 512
NCORES = 8
NB = N // NCORES          # batch rows per core = 16
C = 16                    # chunks per sequence
KS = L // C               # main steps per chunk = 32
S = NB * C                # columns per core = 256
HT = H // 128             # h partition tiles = 4
BLK = 8                   # LN block (main steps)
NBLK = KS // BLK          # 4
FB = BLK * S              # 2048 block columns
SF = HT * S               # 1024 free dim of the packed state


def _bcast_ap(row_ap, parts=128, rep=1):
    """DRAM row AP -> partition-broadcast AP (0-stride over partitions,
    optionally replicated rep times along free)."""
    ap = [[0, parts]]
    if rep > 1:
        ap.append([0, rep])
    ap += [list(d) for d in row_ap.ap]
    return bass.AP(tensor=row_ap.tensor, offset=row_ap.offset, ap=ap)


def build_program(R=12, debug=False, triv_gb=True):
    T = R + KS
    nc = bacc.Bacc("TRN2", target_bir_lowering=False)

    xs_d = nc.declare_dram_parameter("xs", [T, 128, SF], BF16, isOutput=False)
    ms_d = nc.declare_dram_parameter("ms", [T, S], BF16, isOutput=False)
    h0m_d = nc.declare_dram_parameter("h0m", [128, HT * NB], BF16, isOutput=False)
    wih_d = nc.declare_dram_parameter("wih", [HT, 128, 3 * H], BF16, isOutput=False)
    whh_d = nc.declare_dram_parameter("whh", [HT, 128, 3 * H], BF16, isOutput=False)
    brz_d = nc.declare_dram_parameter("brz", [128, 8], F32, isOutput=False)
    bhn_d = nc.declare_dram_parameter("bhn", [128, HT], F32, isOutput=False)
    bin_d = nc.declare_dram_parameter("bin", [128, HT], F32, isOutput=False)
    gam_d = nc.declare_dram_parameter("gam", [128, HT], F32, isOutput=False)
    bet_d = nc.declare_dram_parameter("bet", [128, HT], F32, isOutput=False)
    ind_d = nc.declare_dram_parameter("ind", [128, 16], BF16, isOutput=False)
    sel_d = nc.declare_dram_parameter("sel", [4, 512], BF16, isOutput=False)

    out_d = nc.declare_dram_parameter("out_st", [HT, 128, KS * S], BF16, isOutput=True)
    if debug:
        dbg_names = ["mk0", "r0", "z0", "n0", "hn0", "s1", "s2x", "hnT"]
        dbg_d = {
            nm: nc.declare_dram_parameter(f"dbg_{nm}", [128, SF], BF16,
                                          isOutput=True)
            for nm in dbg_names
        }
    hl_d = nc.declare_dram_parameter("hlast", [128, HT * NB], BF16, isOutput=True)


    with tile.TileContext(nc) as tc:
        with (
            tc.tile_pool(name="const", bufs=1) as cst,
            tc.tile_pool(name="sb", bufs=1) as sb,
            tc.tile_pool(name="rzp", bufs=1, space="PSUM") as rzp,
            tc.tile_pool(name="gxp", bufs=1, space="PSUM") as gxp,
        ):
            # ---- constants (wih first: needed by the t=0 prefill) ----
            x0 = sb.tile([128, SF], BF16, name="xt0", tag="xt", bufs=4)
            nc.sync.dma_start(out=x0, in_=xs_d[0, :, :])
            wih_sb, whh_sb = [], []
            for k in range(HT):
                w1 = cst.tile([128, 3 * H], BF16, name=f"wih_sb{k}", tag=f"wih{k}")
                (nc.sync if k % 2 else nc.scalar).dma_start(
                    out=w1, in_=wih_d[k, :, :])
                wih_sb.append(w1)
            for k in range(HT):
                w2 = cst.tile([128, 3 * H], BF16, name=f"whh_sb{k}", tag=f"whh{k}")
                (nc.sync if k % 2 else nc.scalar).dma_start(
                    out=w2, in_=whh_d[k, :, :])
                whh_sb.append(w2)
            h0m_sb = cst.tile([128, HT * NB], BF16, name="h0m_sb", tag="h0m")
            nc.sync.dma_start(out=h0m_sb, in_=h0m_d[:, :])
            brz_sb = cst.tile([128, 8], F32, name="brz_sb", tag="brz")
            nc.sync.dma_start(out=brz_sb, in_=brz_d[:, :])
            bhn_sb = cst.tile([128, HT], F32, name="bhn_sb", tag="bhn")
            nc.sync.dma_start(out=bhn_sb, in_=bhn_d[:, :])
            bin_sb = cst.tile([128, HT], F32, name="bin_sb", tag="bin")
            nc.sync.dma_start(out=bin_sb, in_=bin_d[:, :])
            gam_sb = cst.tile([128, HT], F32, name="gam_sb", tag="gam")
            nc.sync.dma_start(out=gam_sb, in_=gam_d[:, :])
            bet_sb = cst.tile([128, HT], F32, name="bet_sb", tag="bet")
            nc.sync.dma_start(out=bet_sb, in_=bet_d[:, :])
            ind_sb = cst.tile([128, 16], BF16, name="ind_sb", tag="ind")
            nc.sync.dma_start(out=ind_sb, in_=ind_d[:, :])
            eps_sb = cst.tile([128, 1], F32, name="eps_sb", tag="eps")
            nc.vector.memset(eps_sb, 1e-5)
            sel_sb = cst.tile([4, 512], BF16, name="sel_sb", tag="sel")
            nc.sync.dma_start(out=sel_sb, in_=sel_d[:, :])
            # masks: load once to partition 0, broadcast on Pool in
            # chunks (a monolithic broadcast stalls startup for ~17us)
            ms_all = cst.tile([128, T * S], BF16, name="ms_all", tag="msb")
            nc.sync.dma_start(
                out=ms_all[0:1, :], in_=ms_d[:, :].rearrange("t s -> (t s)"))
            nbc = 8
            csz = (T * S + nbc - 1) // nbc
            csz += csz % 2
            for c in range(nbc):
                lo, hi = c * csz, min((c + 1) * csz, T * S)
                if lo >= hi:
                    break
                nc.gpsimd.partition_broadcast(
                    ms_all[:, lo:hi], ms_all[0:1, lo:hi])

            # ---- initial (zero) state ----
            s_cur = sb.tile([128, SF], BF16, name="s_init", tag="state", bufs=3)
            nc.vector.memset(s_cur, 0.0)

            def load_x(t):
                xt = sb.tile([128, SF], BF16, name=f"xt{t}", tag="xt", bufs=4)
                nc.sync.dma_start(out=xt, in_=xs_d[t, :, :])
                return xt



            def prefill_gx(t, xt):
                # open+close gx_n groups for step t (wih only); X pair
                # [gx8|gx9], [gx10|gx11]; stays in PSUM until step t's u.
                gx_ps = [
                    gxp.tile([128, 512], F32, name=f"gx{t}_{j}", tag=f"gx{j}")
                    for j in range(2)
                ]
                for k4 in range(4):
                    j = 8 + k4
                    oap = gx_ps[k4 // 2][:, (k4 % 2) * 256 : (k4 % 2) * 256 + 256]
                    for k in range(HT):
                        nc.tensor.matmul(
                            oap, wih_sb[k][:, j * 128 : (j + 1) * 128],
                            xt[:, k * 256 : (k + 1) * 256],
                            start=(k == 0), stop=(k == HT - 1))
                return gx_ps

            xt = x0
            gx_ps = prefill_gx(0, xt)

            # LN pipeline state, keyed by block id
            ln = {}
            y_all = y2_all = None

            for t in range(T):
                main = t >= R
                toff = (t - R) % BLK
                blk = (t - R) // BLK
                blk_end = main and toff == BLK - 1

                # LN pipeline stage for earlier blocks this iteration
                stages = [(t - st["te"], b, st) for b, st in list(ln.items())
                          if 1 <= t - st["te"] <= 8]

                # ---- s1 (ACT top): drain the stats PSUM banks ----
                for dt_, b, st in stages:
                    if dt_ == 1:
                        st_mu = sb.tile([4, 512], F32, name=f"stm{b}",
                                        tag="stm", bufs=2)
                        nc.scalar.activation(
                            out=st_mu, in_=st["stp0"][0:4, :],
                            func=AF.Identity, scale=1.0)
                        st_ss = sb.tile([4, 512], F32, name=f"stv{b}",
                                        tag="stv", bufs=2)
                        nc.vector.tensor_copy(st_ss, st["stp1"][0:4, :])
                        st["st_mu"] = st_mu
                        st["st_ss"] = st_ss

                if t + 1 < T:
                    xt_nxt = load_x(t + 1)
                    mk = ms_all[:, (t + 1) * S : (t + 2) * S]

                # ---- s1 (DVE top): musq, dvar ----
                for dt_, b, st in stages:
                    if dt_ == 1:
                        musq = sb.tile([4, 512], F32, name=f"mq{b}", tag="mq",
                                       bufs=2)
                        nc.vector.tensor_mul(
                            musq, st["st_mu"], st["st_mu"])
                        dvar = sb.tile([4, 512], F32, name=f"dv{b}", tag="dv",
                                       bufs=2)
                        nc.vector.scalar_tensor_tensor(
                            out=dvar, in0=st["st_ss"],
                            scalar=float(H), in1=musq,
                            op0=ALU.mult, op1=ALU.subtract)
                        st["dvar"] = dvar

                # ---- s3 (DVE top): recip + bf16 packs ----
                for dt_, b, st in stages:
                    if dt_ == 3:
                        rst = sb.tile([4, 512], F32, name=f"rs{b}", tag="rs",
                                      bufs=2)
                        nc.vector.reciprocal_approx_fast(out=rst, in_=st["sq"])
                        bsm = sb.tile([4, 512], F32, name=f"bs{b}", tag="bs",
                                      bufs=2)
                        nc.vector.scalar_tensor_tensor(
                            out=bsm, in0=st["st_mu"], scalar=1.0 / float(H),
                            in1=rst, op0=ALU.mult, op1=ALU.mult)
                        pkm = sb.tile([4, 512], BF16, name=f"pkm{b}",
                                      tag="pkm", bufs=2)
                        nc.vector.tensor_copy(pkm, bsm)
                        pkr = sb.tile([4, 512], BF16, name=f"pkr{b}",
                                      tag="pkr", bufs=2)
                        nc.vector.tensor_copy(pkr, rst)
                        st["pkm"] = pkm
                        st["pkr"] = pkr
                        st["mu_bc"] = sb.tile([128, FB], BF16, name=f"mubc{b}",
                                              tag="mubc", bufs=1)
                        st["rs_bc"] = sb.tile([128, FB], BF16, name=f"rsbc{b}",
                                              tag="rsbc", bufs=1)



                # ---- PE: all r/z/gh groups open and close within this
                #      iteration (cross-iteration open groups on sliced
                #      tiles miscompile).  Order: independent wih opens
                #      first, state-dependent whh closes mid-stream, the
                #      complete gx prefill for t+1 last. ----
                r_ps = [
                    rzp.tile([128, 512], F32, name=f"r{t}_{j}", tag=f"r{j}")
                    for j in range(2)
                ]
                gh_ps = [
                    rzp.tile([128, 512], F32, name=f"gh{t}_{j}", tag=f"gh{j}")
                    for j in range(2)
                ]
                z_ps = [
                    rzp.tile([128, 512], F32, name=f"z{t}_{j}", tag=f"z{j}")
                    for j in range(2)
                ]

                def wih_open(ps, j4, j0):
                    # opens the j4 slice group (start zeroes the bank's
                    # write-bitmap: no other start may hit this bank until
                    # this group fully closes)
                    j = j0 + j4
                    oap = ps[j4 // 2][:, (j4 % 2) * 256 : (j4 % 2) * 256 + 256]
                    for k in range(HT):
                        nc.tensor.matmul(
                            oap, wih_sb[k][:, j * 128 : (j + 1) * 128],
                            xt[:, k * 256 : (k + 1) * 256],
                            start=(k == 0), stop=False)

                def whh_close(ps, j4, j0):
                    j = j0 + j4
                    oap = ps[j4 // 2][:, (j4 % 2) * 256 : (j4 % 2) * 256 + 256]
                    for k in range(HT):
                        nc.tensor.matmul(
                            oap, whh_sb[k][:, j * 128 : (j + 1) * 128],
                            s_cur[:, k * 256 : (k + 1) * 256],
                            start=False, stop=(k == HT - 1))

                def gh_bank(h):
                    for k4 in (2 * h, 2 * h + 1):
                        j = 8 + k4
                        oap = gh_ps[h][:, (k4 % 2) * 256 : (k4 % 2) * 256 + 256]
                        for k in range(HT):
                            nc.tensor.matmul(
                                oap, whh_sb[k][:, j * 128 : (j + 1) * 128],
                                s_cur[:, k * 256 : (k + 1) * 256],
                                start=(k == 0), stop=(k == HT - 1))

                # independent x-projections first (one open per bank),
                # then per-bank sequential close/open/close
                wih_open(r_ps, 0, 0)
                wih_open(r_ps, 2, 0)
                wih_open(z_ps, 0, 4)
                wih_open(z_ps, 2, 4)
                whh_close(r_ps, 0, 0)
                wih_open(r_ps, 1, 0)
                whh_close(r_ps, 1, 0)
                whh_close(r_ps, 2, 0)
                wih_open(r_ps, 3, 0)
                whh_close(r_ps, 3, 0)
                gh_bank(0)
                gh_bank(1)
                whh_close(z_ps, 0, 4)
                wih_open(z_ps, 1, 4)
                whh_close(z_ps, 1, 4)
                whh_close(z_ps, 2, 4)
                wih_open(z_ps, 3, 4)
                whh_close(z_ps, 3, 4)
                if t + 1 < T:
                    gx_nxt = prefill_gx(t + 1, xt_nxt)

                # ---- ACT: r sigmoids (bf16 out) ----
                r_t = sb.tile([128, SF], BF16, name=f"rt{t}", tag="rt", bufs=2)
                for k in range(HT):
                    nc.scalar.activation(
                        out=r_t[:, k * 256 : (k + 1) * 256],
                        in_=r_ps[k // 2][:, (k % 2) * 256 : (k % 2) * 256 + 256],
                        func=AF.Sigmoid, bias=brz_sb[:, k : k + 1], scale=1.0)

                # ---- DVE: w = (gh + bhn) * r  (per k-tile) ----
                w_t = sb.tile([128, SF], BF16, name=f"wt{t}", tag="wt", bufs=2)
                for k in range(HT):
                    nc.vector.scalar_tensor_tensor(
                        out=w_t[:, k * 256 : (k + 1) * 256],
                        in0=gh_ps[k // 2][:, (k % 2) * 256 : (k % 2) * 256 + 256],
                        scalar=bhn_sb[:, k : k + 1],
                        in1=r_t[:, k * 256 : (k + 1) * 256],
                        op0=ALU.add, op1=ALU.mult)

                # ---- DVE: u = w + gx (per X bank, [128,512]);
                #      Pool cannot read PSUM on TRN2 ----
                u_t = sb.tile([128, SF], BF16, name=f"ut{t}", tag="ut", bufs=2)
                for h in range(2):
                    nc.vector.tensor_add(
                        u_t[:, h * 512 : (h + 1) * 512],
                        w_t[:, h * 512 : (h + 1) * 512], gx_ps[h])

                # ---- ACT: tanh (per k-tile, bias=b_ih_n) / z sigmoids ----
                n_t = sb.tile([128, SF], BF16, name=f"nt{t}", tag="nt", bufs=2)
                z_t = sb.tile([128, SF], BF16, name=f"zt{t}", tag="zt", bufs=2)

                def tanh_k(k):
                    nc.scalar.activation(
                        out=n_t[:, k * 256 : (k + 1) * 256],
                        in_=u_t[:, k * 256 : (k + 1) * 256],
                        func=AF.Tanh, bias=bin_sb[:, k : k + 1], scale=1.0)

                def zsig_k(k):
                    nc.scalar.activation(
                        out=z_t[:, k * 256 : (k + 1) * 256],
                        in_=z_ps[k // 2][:, (k % 2) * 256 : (k % 2) * 256 + 256],
                        func=AF.Sigmoid, bias=brz_sb[:, 4 + k : 5 + k], scale=1.0)

                tanh_k(0)
                tanh_k(1)
                zsig_k(0)
                zsig_k(1)
                tanh_k(2)
                tanh_k(3)
                zsig_k(2)
                zsig_k(3)

                # ---- DVE chain (two [128,512] halves):
                #      t1 = s - n; t1 *= z; hn = t1 + n; s' = hn*mk;
                #      y = hn + x; y2 = y*y ----
                hn = sb.tile([128, SF], BF16, name=f"hn{t}", tag="hn", bufs=2)
                t1 = sb.tile([128, SF], BF16, name=f"t1{t}", tag="t1", bufs=2)
                s_nxt = None
                if t + 1 < T:
                    s_nxt = sb.tile([128, SF], BF16, name=f"s{t + 1}",
                                    tag="state", bufs=3)
                if main and toff == 0:
                    y_all = sb.tile([128, HT * FB], BF16,
                                    name=f"y{blk}", tag="y_all", bufs=2)
                    y2_all = sb.tile([128, HT * FB], BF16,
                                     name=f"y2{blk}", tag="y2_all", bufs=1)

                for h in range(2):
                    sl = slice(h * 512, (h + 1) * 512)
                    nc.vector.tensor_sub(t1[:, sl], s_cur[:, sl], n_t[:, sl])
                    nc.vector.tensor_mul(t1[:, sl], t1[:, sl], z_t[:, sl])
                    nc.vector.tensor_add(hn[:, sl], t1[:, sl], n_t[:, sl])
                    if s_nxt is not None:
                        for k in (2 * h, 2 * h + 1):
                            ksl = slice(k * 256, (k + 1) * 256)
                            nc.vector.tensor_mul(
                                s_nxt[:, ksl], hn[:, ksl], mk)
                if main:
                    for k in range(HT):
                        o = k * FB + toff * S
                        ksl = slice(k * 256, (k + 1) * 256)
                        nc.vector.tensor_add(
                            y_all[:, o : o + S], hn[:, ksl], xt[:, ksl])
                        nc.gpsimd.tensor_mul(
                            y2_all[:, o : o + S],
                            y_all[:, o : o + S], y_all[:, o : o + S])

                if debug and t == 0:
                    pass
                    nc.sync.dma_start(out=dbg_d["r0"][:, :], in_=r_t)
                    nc.sync.dma_start(out=dbg_d["z0"][:, :], in_=z_t)
                    nc.sync.dma_start(out=dbg_d["n0"][:, :], in_=n_t)
                    nc.sync.dma_start(out=dbg_d["hn0"][:, :], in_=hn)
                    nc.sync.dma_start(out=dbg_d["s1"][:, :], in_=s_nxt)
                if debug and t == 1:
                    nc.sync.dma_start(out=dbg_d["s2x"][:, :], in_=s_nxt)
                if debug and t == T - 1:
                    nc.sync.dma_start(out=dbg_d["hnT"][:, :], in_=hn)

                # -- h0 injection at entry to main (chunk-0 columns) --
                if t + 1 == R:
                    inj = s_nxt.rearrange("p (k c) -> p k c", k=HT)[:, :, 0:S:C]
                    nc.vector.tensor_copy(
                        inj, h0m_sb.rearrange("p (k n) -> p k n", k=HT))

                # -- final hidden state (chunk C-1 columns); compact on
                #    DVE first (a strided DMA lowers to 2-byte packets) --
                if t == T - 1:
                    hl = hn.rearrange("p (k c) -> p k c", k=HT)[
                        :, :, C - 1 : S : C]
                    hlc = sb.tile([128, HT * NB], BF16, name="hlc", tag="hlc")
                    nc.vector.tensor_copy(
                        hlc.rearrange("p (k n) -> p k n", k=HT), hl)
                    nc.sync.dma_start(out=hl_d[:, :], in_=hlc)

                # ---- s2 (ACT tail): sqrt (costs 2 act-table loads) ----
                for dt_, b, st in stages:
                    if dt_ == 2:
                        sq = sb.tile([4, 512], F32, name=f"sq{b}", tag="sq",
                                     bufs=2)
                        nc.scalar.activation(
                            out=sq, in_=st["dvar"], func=AF.Sqrt,
                            bias=eps_sb[0:4, :],
                            scale=1.0 / (float(H) * float(H)))
                        st["sq"] = sq

                # ---- yn stages (DVE tail): normalize one k-tile ----
                for dt_, b, st in stages:
                    if 5 <= dt_ <= 8:
                        k = dt_ - 5
                        yn = sb.tile([128, FB], BF16, name=f"yn{b}_{k}",
                                     tag="yn", bufs=3)
                        nc.vector.tensor_mul(
                            yn, st["y_all"][:, k * FB : (k + 1) * FB],
                            st["rs_bc"])
                        nc.vector.tensor_sub(yn, yn, st["mu_bc"])
                        if not triv_gb:
                            nc.gpsimd.tensor_scalar(
                                out=yn, in0=yn,
                                scalar1=gam_sb[:, k : k + 1],
                                scalar2=bet_sb[:, k : k + 1],
                                op0=ALU.mult, op1=ALU.add)
                        (nc.sync if k % 2 else nc.scalar).dma_start(
                            out=out_d[k, :, b * FB : (b + 1) * FB], in_=yn)
                        if k == HT - 1:
                            del ln[b]

                # ---- bc waves (PE tail): broadcast Smu/rs rows to
                #      [128,512] PSUM tiles via selector matmuls, borrowing
                #      the gh and z bank pairs (complete 1-mm groups) ----
                for dt_, b, st in stages:
                    if dt_ in (3, 4):
                        waves = (0, 1) if dt_ == 3 else (2, 3)
                        for i, g in enumerate(waves):
                            tg = ("gh0", "gh1") if i == 0 else ("r0", "r1")
                            mu_ps = rzp.tile([128, 512], F32,
                                             name=f"bcm{b}_{g}", tag=tg[0])
                            rs_ps = rzp.tile([128, 512], F32,
                                             name=f"bcr{b}_{g}", tag=tg[1])
                            nc.tensor.matmul(
                                mu_ps, sel_sb[0:4, g * 128 : (g + 1) * 128],
                                st["pkm"], start=True, stop=True,
                                skip_group_check=True)
                            nc.tensor.matmul(
                                rs_ps, sel_sb[0:4, g * 128 : (g + 1) * 128],
                                st["pkr"], start=True, stop=True,
                                skip_group_check=True)
                            nc.scalar.activation(
                                out=st["mu_bc"][:, g * 512 : (g + 1) * 512],
                                in_=mu_ps, func=AF.Identity, scale=1.0)
                            nc.scalar.activation(
                                out=st["rs_bc"][:, g * 512 : (g + 1) * 512],
                                in_=rs_ps, func=AF.Identity, scale=1.0)

                # ---- blk_end (PE tail): LN column-sum matmuls ----
                if blk_end:
                    stp0 = rzp.tile([128, 512], F32, name=f"st{blk}a",
                                    tag="gh0")
                    stp1 = rzp.tile([128, 512], F32, name=f"st{blk}b",
                                    tag="gh1")
                    for g in range(4):   # Smu rows 0..3 <- y cols g*512..
                        for k in range(HT):
                            nc.tensor.matmul(
                                stp0[0:4, :],
                                ind_sb[:, g * 4 : (g + 1) * 4],
                                y_all[:, k * FB + g * 512 : k * FB + g * 512 + 512],
                                start=(g == 0 and k == 0),
                                stop=(g == 3 and k == HT - 1),
                                skip_group_check=True)
                    for g in range(4):   # Sss rows 0..3 <- y2
                        for k in range(HT):
                            nc.tensor.matmul(
                                stp1[0:4, :],
                                ind_sb[:, g * 4 : (g + 1) * 4],
                                y2_all[:, k * FB + g * 512 : k * FB + g * 512 + 512],
                                start=(g == 0 and k == 0),
                                stop=(g == 3 and k == HT - 1),
                                skip_group_check=True)
                    ln[blk] = {"te": t, "stp0": stp0, "stp1": stp1,
                               "y_all": y_all}

                if t + 1 < T:
                    s_cur = s_nxt
                    xt = xt_nxt
                    gx_ps = gx_nxt

            # ---- tail: finish LN for the last block(s) ----
            for b in sorted(ln):
                st = ln[b]
                st_mu = sb.tile([4, 512], F32, name=f"tstm{b}", tag="stm",
                                bufs=2)
                nc.scalar.activation(
                    out=st_mu, in_=st["stp0"][0:4, :], func=AF.Identity,
                    scale=1.0)
                st_ss = sb.tile([4, 512], F32, name=f"tstv{b}", tag="stv",
                                bufs=2)
                nc.scalar.activation(
                    out=st_ss, in_=st["stp1"][0:4, :], func=AF.Identity,
                    scale=1.0)
                musq = sb.tile([4, 512], F32, name=f"tmq{b}", tag="mq", bufs=2)
                nc.vector.tensor_mul(musq, st_mu, st_mu)
                dvar = sb.tile([4, 512], F32, name=f"tdv{b}", tag="dv", bufs=2)
                nc.vector.scalar_tensor_tensor(
                    out=dvar, in0=st_ss, scalar=float(H), in1=musq,
                    op0=ALU.mult, op1=ALU.subtract)
                sq = sb.tile([4, 512], F32, name=f"tsq{b}", tag="sq", bufs=2)
                nc.scalar.activation(
                    out=sq, in_=dvar, func=AF.Sqrt,
                    bias=eps_sb[0:4, :],
                    scale=1.0 / (float(H) * float(H)))
                rst = sb.tile([4, 512], F32, name=f"trs{b}", tag="rs", bufs=2)
                nc.vector.reciprocal_approx_fast(out=rst, in_=sq)
                bsm = sb.tile([4, 512], F32, name=f"tbs{b}", tag="bs", bufs=2)
                nc.vector.scalar_tensor_tensor(
                    out=bsm, in0=st_mu, scalar=1.0 / float(H),
                    in1=rst, op0=ALU.mult, op1=ALU.mult)
                pkm = sb.tile([4, 512], BF16, name=f"tpkm{b}", tag="pkm",
                              bufs=2)
                nc.vector.tensor_copy(pkm, bsm)
                pkr = sb.tile([4, 512], BF16, name=f"tpkr{b}", tag="pkr",
                              bufs=2)
                nc.vector.tensor_copy(pkr, rst)
                mu_bc = sb.tile([128, FB], BF16, name=f"tmubc{b}", tag="mubc",
                                bufs=1)
                rs_bc = sb.tile([128, FB], BF16, name=f"trsbc{b}", tag="rsbc",
                                bufs=1)
                tags = [("gh0", "gh1"), ("z0", "z1"), ("r0", "r1"),
                        ("gx0", "gx1")]
                for g in range(4):
                    tg = tags[g]
                    pool = gxp if tg[0].startswith("gx") else rzp
                    mu_ps = rzp.tile([128, 512], F32, name=f"tbm{b}_{g}",
                                     tag=tg[0]) if tg[0] != "gx0" else                         gxp.tile([128, 512], F32, name=f"tbm{b}_{g}",
                                 tag=tg[0])
                    rs_ps = rzp.tile([128, 512], F32, name=f"tbr{b}_{g}",
                                     tag=tg[1]) if tg[1] != "gx1" else                         gxp.tile([128, 512], F32, name=f"tbr{b}_{g}",
                                 tag=tg[1])
                    nc.tensor.matmul(
                        mu_ps, sel_sb[0:4, g * 128 : (g + 1) * 128], pkm,
                        start=True, stop=True, skip_group_check=True)
                    nc.tensor.matmul(
                        rs_ps, sel_sb[0:4, g * 128 : (g + 1) * 128], pkr,
                        start=True, stop=True, skip_group_check=True)
                    nc.scalar.activation(
                        out=mu_bc[:, g * 512 : (g + 1) * 512], in_=mu_ps,
                        func=AF.Identity, scale=1.0)
                    nc.scalar.activation(
                        out=rs_bc[:, g * 512 : (g + 1) * 512], in_=rs_ps,
                        func=AF.Identity, scale=1.0)
                for k in range(HT):
                    yn = sb.tile([128, FB], BF16, name=f"tyn{b}_{k}",
                                 tag="yn", bufs=3)
                    nc.vector.tensor_mul(
                        yn, st["y_all"][:, k * FB : (k + 1) * FB], rs_bc)
                    nc.vector.tensor_sub(yn, yn, mu_bc)
                    if not triv_gb:
                        nc.gpsimd.tensor_scalar(
                            out=yn, in0=yn,
                            scalar1=gam_sb[:, k : k + 1],
                            scalar2=bet_sb[:, k : k + 1],
                            op0=ALU.mult, op1=ALU.add)
                    (nc.sync if k % 2 else nc.scalar).dma_start(
                        out=out_d[k, :, b * FB : (b + 1) * FB], in_=yn)
    nc.compile()
    return nc


def stage_inputs(input, h, is_initial, W_ih, W_hh, b_ih, b_hh, gamma, beta, R):
    """Host-side sharding/staging. Returns per-core input maps."""
    import ml_dtypes

    T = R + KS
    x = np.asarray(input, np.float32)
    h0 = np.asarray(h, np.float32)
    ii = np.asarray(is_initial).reshape(N, L)
    W_ih = np.asarray(W_ih, np.float32)
    W_hh = np.asarray(W_hh, np.float32)
    b_ih = np.asarray(b_ih, np.float32)
    b_hh = np.asarray(b_hh, np.float32)
    gamma = np.asarray(gamma, np.float32)
    beta = np.asarray(beta, np.float32)

    def bf(a):
        return np.ascontiguousarray(np.asarray(a, np.float32)).astype(
            ml_dtypes.bfloat16)

    mask = 1.0 - ii.astype(np.float32)  # [N, L]

    # l index per (c, t): warm-up reads the R steps before the chunk;
    # chunk 0's warm-up reads l in [KS-R, KS) (discarded garbage).
    l_for = np.empty((C, T), np.int64)
    for c in range(C):
        for t in range(T):
            l = c * KS + (t - R)
            l_for[c, t] = l if l >= 0 else l + KS

    wihT = np.ascontiguousarray(W_ih.T.reshape(HT, 128, 3 * H))
    whhT = np.ascontiguousarray(W_hh.T.reshape(HT, 128, 3 * H))
    brz = (b_ih + b_hh)[: 2 * H].reshape(8, 128).T.copy()        # [128, 8]
    bhn = b_hh[2 * H :].reshape(HT, 128).T.copy()                # [128, 4]
    binn = b_ih[2 * H :].reshape(HT, 128).T.copy()
    gam = gamma.reshape(HT, 128).T.copy()
    bet = beta.reshape(HT, 128).T.copy()
    # indicator stationary: group g (of 4) is a [128, 4] tile whose
    # column g is all-ones (routes a column-sum into PSUM partition g)
    ind = np.zeros((128, 16), np.float32)
    for g in range(4):
        ind[:, g * 4 + g] = 1.0
    # selector for the PSUM->all-partitions broadcast matmuls:
    # sel[c, g*128+po] = 1 iff c == g
    sel = np.zeros((4, 512), np.float32)
    for g in range(4):
        sel[g, g * 128 : (g + 1) * 128] = 1.0

    in_maps = []
    for core in range(NCORES):
        n0 = core * NB
        xc = x[n0 : n0 + NB]              # [NB, L, H]
        xg = xc[:, l_for, :]              # [NB, C, T, H]
        # xs2[t, p, k*S + s] with s = n*C + c, h = k*128 + p
        xs2 = np.ascontiguousarray(
            xg.transpose(2, 3, 0, 1)      # [T, H, NB, C]
            .reshape(T, HT, 128, S)
            .transpose(0, 2, 1, 3)        # [T, 128, HT, S]
            .reshape(T, 128, SF))
        mg = mask[n0 : n0 + NB][:, l_for]  # [NB, C, T]
        ms = np.ascontiguousarray(mg.transpose(2, 0, 1).reshape(T, S))
        m0 = mask[n0 : n0 + NB, 0]         # [NB]
        h0c = h0[n0 : n0 + NB] * m0[:, None]     # [NB, H]
        # h0m[p, k*NB + n] = h0c[n, k*128+p]
        h0m = np.ascontiguousarray(
            h0c.reshape(NB, HT, 128).transpose(2, 1, 0).reshape(128, HT * NB))
        in_maps.append({
            "xs": bf(xs2), "ms": bf(ms), "h0m": bf(h0m),
            "wih": bf(wihT), "whh": bf(whhT),
            "brz": brz, "bhn": bhn, "bin": binn,
            "gam": gam, "bet": bet, "ind": bf(ind), "sel": bf(sel),
        })
    return in_maps


def required_warmup(is_initial):
    """Max distance from a chunk boundary back to the latest reset."""
    ii = np.asarray(is_initial).reshape(N, L)
    need = 0
    for c in range(1, C):
        start = c * KS
        sub = ii[:, :start]
        for n in range(N):
            nz = np.nonzero(sub[n])[0]
            gap = start - nz[-1] if len(nz) else start
            need = max(need, gap)
    return need


def unstage_outputs(results):
    out = np.empty((N, L, H), np.float32)
    h_last = np.empty((N, H), np.float32)
    for core in range(NCORES):
        n0 = core * NB
        st = np.asarray(results[core]["out_st"], np.float32)  # [HT,128,KS*S]
        o = st.reshape(HT, 128, KS, NB, C).transpose(3, 4, 2, 0, 1)
        out[n0 : n0 + NB] = o.reshape(NB, L, H)
        hl = np.asarray(results[core]["hlast"], np.float32)   # [128, HT*NB]
        h_last[n0 : n0 + NB] = (
            hl.reshape(128, HT, NB).transpose(2, 1, 0).reshape(NB, H))
    h_exp = np.broadcast_to(h_last[:, None, :], (N, L, H)).copy()
    return out, h_exp


_PROGRAM_CACHE = {}


def kernel(input, h, is_initial, W_ih, W_hh, b_ih, b_hh, gamma, beta):
    need = required_warmup(is_initial)
    R = max(12, int(need))
    triv = bool(
        np.all(np.asarray(gamma) == 1.0) and np.all(np.asarray(beta) == 0.0))
    key = (R, triv)
    if key not in _PROGRAM_CACHE:
        _PROGRAM_CACHE[key] = build_program(R, triv_gb=triv)
    nc = _PROGRAM_CACHE[key]
    in_maps = stage_inputs(
        input, h, is_initial, W_ih, W_hh, b_ih, b_hh, gamma, beta, R)
    res = run_bass_kernel_spmd(nc, in_maps, list(range(NCORES))).results
    return unstage_outputs(res)
